# revision 18
# baseline (speedup 1.0000x reference)
"""Trainium2 Bass kernel for sparse_attention problem nn_CAMD_73229192397362.

v4 precision model (HW-validated: fp32r = round-to-nearest ~11-bit, but
the tolerance needs ~15+ bits on every path feeding the 7e4-magnitude
accumulations):
  - Both MLPs, band S^T, knat transposes, H snapshots and prefix-zo run
    in fp32.
  - The prefix chain K^T V runs as THREE fp32r matmuls per chunk
    (Kr Vr + Kr Ve + Ke Vr) with exact splits: V split on host,
    K split on-chip from the fp32 knat (round-copy + subtract).
  - The band zo runs fp32r on the fp32-exact masked S (smt) and Vr; its
    residuals are per-key random and average out over the band.

Structure per core (8 = 4 modalities x 2 query half-sets):
  stacked s4 layout (chunk c -> partitions 32*(c%4), cols 128*(c//4));
  block-diag 128-contract MLPs; rotated (tile_position) 32-contract
  band S^T and knat transposes; per-tile prefix H folded in via rotated
  fp32 prefix-zo into 4 PSUM banks (zoP), combined with the band zo
  accumulator (zoB) on DVE at the end of each 512-query group.
"""

import numpy as np

import concourse.bass as bass
from concourse.bacc import Bacc
import concourse.mybir as mybir
from concourse.tile import TileContext
from concourse.bass_utils import run_bass_kernel_spmd

T = 8192
D = 32
TQ = 4096
NT = TQ // 128
NCH = T // 128
NG = NT // 4
F32 = mybir.dt.float32
F32R = mybir.dt.float32r
AF = mybir.ActivationFunctionType
OP = mybir.AluOpType

# packed input column maps
W32_COLS = 384 + 384 + 3 + 3 + 32 + NCH   # wq | wk | bq | bk | id4 | t2p
WR_COLS = 4 * NCH                         # v4n: per chunk [vr0 vr1 ve0 ve1]


def _s4(xT):
    """(32, N) -> (128, N//4): 128-col chunk c -> partitions 32*(c%4),
    cols 128*(c//4)."""
    d, N = xT.shape
    nch = N // 128
    out = np.zeros((128, N // 4), dtype=xT.dtype)
    for c in range(nch):
        out[32 * (c % 4):32 * (c % 4) + 32,
            128 * (c // 4):128 * (c // 4) + 128] = xT[:, 128 * c:128 * c + 128]
    return out


def _band_meta(t1_all, t2_all):
    w_raw = np.full(NT, T, dtype=np.int64)
    for t1 in t1_all:
        for t2 in t2_all:
            r_min = np.searchsorted(t2, t1[::128], side="right")
            w_raw = np.minimum(w_raw, (r_min // 128) * 128)
    e = np.zeros(NT, dtype=np.int64)
    for t1 in t1_all:
        for t2 in t2_all:
            r_max = np.searchsorted(t2, t1[127::128], side="right")
            e = np.maximum(e, (r_max + 127) // 128)
    wc = w_raw // 128
    e = np.maximum(e, wc + 1)
    e = np.minimum(np.maximum.accumulate(e), NCH)
    wc = np.minimum(wc, e - 1)
    assert np.all(np.diff(wc) >= 0) and np.all(np.diff(e) >= 0)
    for t1 in t1_all:
        for t2 in t2_all:
            r_min = np.searchsorted(t2, t1[::128], side="right")
            r_max = np.searchsorted(t2, t1[127::128], side="right")
            assert np.all(wc * 128 <= r_min) and np.all(r_max <= e * 128)
    return [int(x) for x in wc], [int(x) for x in e]


def _pieces_meta(wc, e, t1_all, t2_all):
    pieces = []
    for c in range(NCH):
        tiles = [I for I in range(NT) if wc[I] <= c < e[I]]
        if not tiles:
            continue
        lo, ihi = tiles[0], tiles[-1] + 1
        qlo = 128 * lo
        qmin = TQ
        for t1 in t1_all:
            for t2 in t2_all:
                qmin = min(qmin, int(np.searchsorted(t1, t2[128 * c])))
        qlo = max(qlo, (qmin // 64) * 64)
        qlo = min(qlo, 128 * ihi - 64)
        pieces.append((c, qlo, lo, ihi))
    return pieces


def _build(wc, e, pieces):
    nc = Bacc("TRN2")

    xk = nc.dram_tensor("xk", [128, T // 4], F32, kind="ExternalInput")
    xq = nc.dram_tensor("xq", [128, TQ // 4], F32, kind="ExternalInput")
    wp32 = nc.dram_tensor("wp32", [128, W32_COLS], F32, kind="ExternalInput")
    wpr = nc.dram_tensor("wpr", [128, WR_COLS], F32R, kind="ExternalInput")
    t1 = nc.dram_tensor("t1", [1, TQ], F32, kind="ExternalInput")
    out = nc.dram_tensor("out", [2, TQ], F32, kind="ExternalOutput")
    hdbg = nc.dram_tensor("hdbg", [32, 4 * NT], F32, kind="ExternalOutput")

    maxw = max(wc)

    gparts = {g: [] for g in range(NG)}
    for idx, (c, qlo, lo, ihi) in enumerate(pieces):
        for g in range((qlo // 512), (ihi * 128 - 1) // 512 + 1):
            a = max(qlo, 512 * g)
            b = min(128 * ihi, 512 * g + 512)
            gparts[g].append((idx, a, b))

    with TileContext(nc) as tc:
        with tc.tile_pool(name="cst", bufs=1) as cst, \
             tc.tile_pool(name="big", bufs=1) as big:

            wp32_s = cst.tile([128, W32_COLS], F32)
            wpr_s = cst.tile([128, WR_COLS], F32R)
            wq_s = wp32_s[:, 0:384]
            wk_s = wp32_s[:, 384:768]
            bq_s = wp32_s[:, 768:771]
            bk_s = wp32_s[:, 771:774]
            id4_s = wp32_s[:, 774:806]
            t2p_s = wp32_s[:, 806:806 + NCH]
            v4n_s = wpr_s[:, 0:4 * NCH]

            t1b_s = big.tile([128, TQ], F32, tag="t1b")
            xk_a = big.tile([128, T // 8], F32, tag="xka")
            xk_b = big.tile([128, T // 8], F32, tag="xkb")
            xq_s = big.tile([128, TQ // 4], F32, tag="xq")
            kt_s = big.tile([128, T // 4], F32, tag="kt")
            qts32 = big.tile([128, TQ // 4], F32, tag="qts32")
            qrep32 = big.tile([128, TQ], F32, tag="qrep32")
            knr = big.tile([128, 32 * NCH], F32R, tag="knr")
            kne = big.tile([128, 32 * NCH], F32R, tag="kne")
            hsball = cst.tile([32, 4 * NT], F32)
            hsbrep = cst.tile([128, 4 * NT], F32)
            zsb = cst.tile([2, TQ], F32)

            nc.sync.dma_start(wp32_s[:], wp32[:])
            nc.sync.dma_start(xq_s[:], xq[:])
            half = T // 8
            nc.scalar.dma_start(xk_a[:], xk[:, 0:half])
            nc.scalar.dma_start(xk_b[:], xk[:, half:])
            nc.sync.dma_start(wpr_s[:], wpr[:])
            nc.gpsimd.dma_start(
                t1b_s[:], t1[0:1, :].partition_broadcast(128))

            # ---------------- MLPs (block-diag 128-contract) -------------
            with tc.tile_pool(name="mlp", bufs=3, space="PSUM") as mlp, \
                 tc.tile_pool(name="hbuf", bufs=2) as hbuf:

                def run_mlp(x_parts, w_s, b_s, ngrp, dst, dt):
                    h_prev = None
                    for l in range(3):
                        h_next = dst if l == 2 else hbuf.tile(
                            [128, ngrp * 512], dt, tag=f"h{ngrp}",
                            name=f"h{ngrp}_{l}")
                        for G in range(ngrp):
                            if l == 0:
                                npart = len(x_parts)
                                gper = ngrp // npart
                                src_ap = x_parts[G // gper][
                                    :, 512 * (G % gper):512 * (G % gper) + 512]
                            else:
                                src_ap = h_prev[:, 512 * G:512 * G + 512]
                            pt = mlp.tile([128, 512], F32, tag="mlp")
                            nc.tensor.matmul(
                                pt[:], w_s[:, 128 * l:128 * l + 128],
                                src_ap,
                                start=True, stop=True)
                            o = h_next[:, 512 * G:512 * G + 512]
                            if G % 2 == 0:
                                if l < 2:
                                    nc.scalar.activation(
                                        o, pt[:], AF.Relu, bias=b_s[:, l:l + 1])
                                else:
                                    nc.scalar.activation(
                                        o, pt[:], AF.Identity,
                                        bias=b_s[:, l:l + 1])
                            else:
                                if l < 2:
                                    nc.vector.tensor_scalar(
                                        o, pt[:], b_s[:, l:l + 1], 0.0,
                                        OP.add, OP.max)
                                else:
                                    nc.vector.tensor_scalar(
                                        o, pt[:], b_s[:, l:l + 1], None,
                                        OP.add)
                        h_prev = h_next

                run_mlp([xq_s], wq_s, bq_s, 2, qts32, F32)
                run_mlp([xk_a, xk_b], wk_s, bk_s, 4, kt_s, F32)

            # replicated flat Q^T (fp32), issued off-ACT
            for b in range(4):
                for k in range(4):
                    nc.gpsimd.dma_start(
                        qrep32[32 * b:32 * b + 32, :].rearrange(
                            "d (t c) -> d t c", c=128)[:, k::4, :],
                        qts32[32 * k:32 * k + 32, :].rearrange(
                            "d (t c) -> d t c", c=128))

            # knat transposes (fp32) + exact split into knr/kne (fp32r)
            with tc.tile_pool(name="knT", bufs=1, space="PSUM") as knT:
                knrv = knr[:, :].rearrange("p (c d) -> p c d", d=32)
                knev = kne[:, :].rearrange("p (c d) -> p c d", d=32)
                for half in range(2):
                    pts = [knT.tile([128, 256], F32, tag=f"knT{q}",
                                    name=f"knT{q}") for q in range(4)]
                    for i in range(8):
                        for q in range(4):
                            col = 8 * half + i
                            nc.tensor.matmul(
                                pts[q][:, 32 * i:32 * i + 32],
                                kt_s[32 * q:32 * q + 32,
                                     128 * col:128 * col + 128],
                                id4_s[32 * q:32 * q + 32, :],
                                start=True, stop=True,
                                tile_position=(32 * q, 0))
                    for q in range(4):
                        pv = pts[q][:].rearrange("p (c d) -> p c d", d=32)
                        orr = knrv[:, q + 4 * 8 * half::4, :][:, 0:8, :]
                        oe = knev[:, q + 4 * 8 * half::4, :][:, 0:8, :]
                        nc.scalar.activation(orr, pv, AF.Copy)
                        nc.vector.scalar_tensor_tensor(
                            oe, orr.bitcast(F32), -1.0, pv,
                            OP.mult, OP.add)

            # prefix chain (fp32r) + per-tile H snapshots (fp32) -> hsball
            with tc.tile_pool(name="hps", bufs=3, space="PSUM") as hps:
                prev = 0
                for I in range(NT):
                    w = wc[I]
                    dst = hsball[:, 4 * I:4 * I + 4]
                    if w > prev:
                        dps = hps.tile([32, 4], F32, tag="dh")
                        for c in range(prev, w):
                            nc.tensor.matmul(
                                dps[:], knr[:, 32 * c:32 * c + 32],
                                v4n_s[:, 4 * c:4 * c + 4],
                                start=(c == prev), stop=False)
                            nc.tensor.matmul(
                                dps[:, 0:2], kne[:, 32 * c:32 * c + 32],
                                v4n_s[:, 4 * c:4 * c + 2],
                                start=False, stop=(c == w - 1),
                                skip_group_check=True)
                        if I == 0:
                            nc.vector.tensor_copy(dst, dps[:])
                        else:
                            nc.vector.tensor_tensor(
                                dst, hsball[:, 4 * I - 4:4 * I],
                                dps[:], OP.add)
                        prev = w
                    elif I == 0:
                        nc.vector.memset(dst, 0)
                    else:
                        nc.vector.tensor_copy(
                            dst, hsball[:, 4 * I - 4:4 * I])

            # replicate H table (fp32) to all 4 partition blocks
            for q in range(4):
                nc.gpsimd.dma_start(hsbrep[32 * q:32 * q + 32, :], hsball[:])


            # ---------------- band ----------------
            with tc.tile_pool(name="stp", bufs=3, space="PSUM") as stp, \
                 tc.tile_pool(name="zob", bufs=1, space="PSUM") as zob, \
                 tc.tile_pool(name="zop", bufs=1, space="PSUM") as zop, \
                 tc.tile_pool(name="smp", bufs=10) as smp, \
             tc.tile_pool(name="zps", bufs=2) as zpsp:

                # rotated fp32 prefix-zo straight into zoB rows 0:4
                zoBs = {}

                def emit_prefix_batch(gs):
                    for g in gs:
                        zoBs[g] = zob.tile([4, 512], F32, tag=f"zoB{g % 4}",
                                           name=f"zoB{g % 4}")
                    for t in range(4):
                        for g in gs:
                            I = 4 * g + t
                            qq = g % 4
                            nc.tensor.matmul(
                                zoBs[g][:, 128 * t:128 * t + 128],
                                hsbrep[32 * qq:32 * qq + 32, 4 * I:4 * I + 4],
                                qrep32[32 * qq:32 * qq + 32,
                                       128 * I:128 * I + 128],
                                start=(t == 0), stop=False,
                                tile_position=(32 * qq, 0),
                                skip_group_check=True)

                emit_prefix_batch(range(0, 4))
                made = {}
                for g in range(NG):
                    if g == 4:
                        emit_prefix_batch(range(4, 8))
                    zoB = zoBs[g]
                    for (idx, a, b) in gparts[g]:
                        if idx in made:
                            continue
                        c, qlo, lo, ihi = pieces[idx]
                        wd = 128 * ihi - qlo
                        q = c % 4
                        stb = stp.tile([128, 512], F32, tag="st")
                        nc.tensor.matmul(
                            stb[:, 0:wd],
                            kt_s[32 * q:32 * q + 32,
                                 128 * (c // 4):128 * (c // 4) + 128],
                            qrep32[32 * q:32 * q + 32, qlo:128 * ihi],
                            start=True, stop=True,
                            tile_position=(32 * q, 0))
                        smt = smp.tile([128, 512], F32R, tag="smt")
                        nc.vector.scalar_tensor_tensor(
                            smt[:, 0:wd], t1b_s[:, qlo:128 * ihi],
                            t2p_s[:, c:c + 1], stb[:, 0:wd],
                            OP.is_ge, OP.mult)
                        made[idx] = smt
                    nparts = len(gparts[g])
                    assert nparts > 0
                    for i, (idx, a, b) in enumerate(gparts[g]):
                        c, qlo, lo, ihi = pieces[idx]
                        smt = made[idx]
                        nc.tensor.matmul(
                            zoB[:, a - 512 * g:b - 512 * g],
                            v4n_s[:, 4 * c:4 * c + 4],
                            smt[:, a - qlo:b - qlo],
                            start=False, stop=(i == nparts - 1),
                            skip_group_check=True)
                    zb4 = zpsp.tile([4, 512], F32, tag="zb4")
                    nc.scalar.activation(zb4[:], zoB[:], AF.Copy)
                    zbs = zpsp.tile([2, 512], F32, tag="zbs")
                    nc.gpsimd.dma_start(zbs[:], zb4[2:4, :])
                    nc.vector.tensor_tensor(
                        zsb[:, 512 * g:512 * g + 512], zb4[0:2, :], zbs[:],
                        OP.add)

            nc.sync.dma_start(out[:], zsb[:])
            nc.sync.dma_start(hdbg[:], hsball[:])
    nc.finalize()
    return nc


_CACHE = {}
LAST_RESULTS = None


def kernel(m1, m2, m3, m4, Wq, bq, Wk, bk):
    mods = [np.asarray(m)[0, 0].astype(np.float32) for m in (m1, m2, m3, m4)]
    Wq, bq, Wk, bk = (np.asarray(a, dtype=np.float32) for a in (Wq, bq, Wk, bk))
    t2s = [m[:, -1].copy() for m in mods]
    t1g = mods[0][:, -1].copy()

    def qsel(h):
        idx = np.arange(TQ)
        gt = 2 * (idx // 128) + h
        return gt * 128 + (idx % 128)

    sels = [qsel(0), qsel(1)]
    t1_locals = [t1g[s] for s in sels]
    wc, e = _band_meta(t1_locals, t2s)
    pieces = _pieces_meta(wc, e, t1_locals, t2s)

    key = (tuple(wc), tuple(e), tuple(p[1] for p in pieces))
    if key not in _CACHE:
        _CACHE[key] = _build(wc, e, pieces)
    nc = _CACHE[key]

    def chop11(x):
        # zero low 12 mantissa bits: exactly representable in fp32r (11b)
        xm = np.ascontiguousarray(x, np.float32).view(np.uint32)
        return (xm & np.uint32(0xFFFFF000)).view(np.float32).copy()

    def blockdiag(W):
        out = np.zeros((128, 384), dtype=np.float32)
        for l in range(3):
            for k in range(4):
                out[32 * k:32 * k + 32,
                    128 * l + 32 * k:128 * l + 32 * k + 32] = W[l]
        return out

    bq_in = np.tile(bq.T, (4, 1)).astype(np.float32)
    bk_in = np.tile(bk.T, (4, 1)).astype(np.float32)
    id4_in = np.tile(np.eye(32, dtype=np.float32), (4, 1))

    in_maps = []
    for core in range(8):
        mod, h = core // 2, core % 2
        x = mods[mod]
        t2 = t2s[mod]
        xk_in = _s4(np.ascontiguousarray(x.T))
        xq_l = mods[0][sels[h]]
        xq_in = _s4(np.ascontiguousarray(xq_l.T))
        t1_in = t1_locals[h].reshape(1, TQ).astype(np.float32)
        t2p_in = np.ascontiguousarray(t2.reshape(NCH, 128).T)
        v2n_in = np.ascontiguousarray(
            x[:, :2].reshape(NCH, 128, 2).transpose(1, 0, 2)
            .reshape(128, 2 * NCH))
        v2nr_in = chop11(v2n_in)
        v2ne_in = v2n_in - v2nr_in
        v4n_in = np.zeros((128, 4 * NCH), dtype=np.float32)
        v4n_in[:, 0::4] = v2nr_in[:, 0::2]
        v4n_in[:, 1::4] = v2nr_in[:, 1::2]
        v4n_in[:, 2::4] = v2ne_in[:, 0::2]
        v4n_in[:, 3::4] = v2ne_in[:, 1::2]
        wp32_in = np.concatenate(
            [blockdiag(Wq), blockdiag(Wk), bq_in, bk_in, id4_in, t2p_in],
            axis=1)
        wpr_in = v4n_in
        in_maps.append({
            "xk": xk_in, "xq": xq_in, "wp32": wp32_in, "wpr": wpr_in,
            "t1": t1_in,
        })

    import os as _os
    trace = bool(_os.environ.get("KERNEL_TRACE"))
    res = run_bass_kernel_spmd(nc, in_maps, core_ids=list(range(8)),
                               trace=trace)
    global LAST_RESULTS
    LAST_RESULTS = res

    y = np.zeros((T, 2), dtype=np.float32)
    for core in range(8):
        mod, h = core // 2, core % 2
        zt = res.results[core]["out"]
        y[sels[h]] += zt.T
    return y[None, :, :]


# revision 20
# speedup vs baseline: 1.0577x; 1.0577x over previous
"""Trainium2 Bass kernel for sparse_attention problem nn_CAMD_73229192397362.

v4 precision model (HW-validated: fp32r = round-to-nearest ~11-bit, but
the tolerance needs ~15+ bits on every path feeding the 7e4-magnitude
accumulations):
  - Both MLPs, band S^T, knat transposes, H snapshots and prefix-zo run
    in fp32.
  - The prefix chain K^T V runs as THREE fp32r matmuls per chunk
    (Kr Vr + Kr Ve + Ke Vr) with exact splits: V split on host,
    K split on-chip from the fp32 knat (round-copy + subtract).
  - The band zo runs fp32r on the fp32-exact masked S (smt) and Vr; its
    residuals are per-key random and average out over the band.

Structure per core (8 = 4 modalities x 2 query half-sets):
  stacked s4 layout (chunk c -> partitions 32*(c%4), cols 128*(c//4));
  block-diag 128-contract MLPs; rotated (tile_position) 32-contract
  band S^T and knat transposes; per-tile prefix H folded in via rotated
  fp32 prefix-zo into 4 PSUM banks (zoP), combined with the band zo
  accumulator (zoB) on DVE at the end of each 512-query group.
"""

import numpy as np

import concourse.bass as bass
from concourse.bacc import Bacc
import concourse.mybir as mybir
from concourse.tile import TileContext
from concourse.bass_utils import run_bass_kernel_spmd

T = 8192
D = 32
TQ = 4096
NT = TQ // 128
NCH = T // 128
NG = NT // 4
F32 = mybir.dt.float32
F32R = mybir.dt.float32r
AF = mybir.ActivationFunctionType
OP = mybir.AluOpType

# packed input column maps
W32_COLS = 384 + 384 + 3 + 3 + 32 + NCH   # wq | wk | bq | bk | id4 | t2p
WR_COLS = 4 * NCH                         # v4n: per chunk [vr0 vr1 ve0 ve1]


def _s4(xT):
    """(32, N) -> (128, N//4): 128-col chunk c -> partitions 32*(c%4),
    cols 128*(c//4)."""
    d, N = xT.shape
    nch = N // 128
    out = np.zeros((128, N // 4), dtype=xT.dtype)
    for c in range(nch):
        out[32 * (c % 4):32 * (c % 4) + 32,
            128 * (c // 4):128 * (c // 4) + 128] = xT[:, 128 * c:128 * c + 128]
    return out


def _band_meta(t1_all, t2_all):
    w_raw = np.full(NT, T, dtype=np.int64)
    for t1 in t1_all:
        for t2 in t2_all:
            r_min = np.searchsorted(t2, t1[::128], side="right")
            w_raw = np.minimum(w_raw, (r_min // 128) * 128)
    e = np.zeros(NT, dtype=np.int64)
    for t1 in t1_all:
        for t2 in t2_all:
            r_max = np.searchsorted(t2, t1[127::128], side="right")
            e = np.maximum(e, (r_max + 127) // 128)
    wc = w_raw // 128
    e = np.maximum(e, wc + 1)
    e = np.minimum(np.maximum.accumulate(e), NCH)
    wc = np.minimum(wc, e - 1)
    assert np.all(np.diff(wc) >= 0) and np.all(np.diff(e) >= 0)
    for t1 in t1_all:
        for t2 in t2_all:
            r_min = np.searchsorted(t2, t1[::128], side="right")
            r_max = np.searchsorted(t2, t1[127::128], side="right")
            assert np.all(wc * 128 <= r_min) and np.all(r_max <= e * 128)
    return [int(x) for x in wc], [int(x) for x in e]


def _pieces_meta(wc, e, t1_all, t2_all):
    pieces = []
    for c in range(NCH):
        tiles = [I for I in range(NT) if wc[I] <= c < e[I]]
        if not tiles:
            continue
        lo, ihi = tiles[0], tiles[-1] + 1
        qlo = 128 * lo
        qmin = TQ
        for t1 in t1_all:
            for t2 in t2_all:
                qmin = min(qmin, int(np.searchsorted(t1, t2[128 * c])))
        qlo = max(qlo, (qmin // 64) * 64)
        qlo = min(qlo, 128 * ihi - 64)
        pieces.append((c, qlo, lo, ihi))
    return pieces


def _build(wc, e, pieces):
    nc = Bacc("TRN2")

    xk = nc.dram_tensor("xk", [128, T // 4], F32, kind="ExternalInput")
    xq = nc.dram_tensor("xq", [128, TQ // 4], F32, kind="ExternalInput")
    wp32 = nc.dram_tensor("wp32", [128, W32_COLS], F32, kind="ExternalInput")
    wpr = nc.dram_tensor("wpr", [128, WR_COLS], F32R, kind="ExternalInput")
    t1 = nc.dram_tensor("t1", [1, TQ], F32, kind="ExternalInput")
    out = nc.dram_tensor("out", [2, TQ], F32, kind="ExternalOutput")
    hdbg = nc.dram_tensor("hdbg", [32, 4 * NT], F32, kind="ExternalOutput")

    maxw = max(wc)

    gparts = {g: [] for g in range(NG)}
    for idx, (c, qlo, lo, ihi) in enumerate(pieces):
        for g in range((qlo // 512), (ihi * 128 - 1) // 512 + 1):
            a = max(qlo, 512 * g)
            b = min(128 * ihi, 512 * g + 512)
            gparts[g].append((idx, a, b))

    with TileContext(nc) as tc:
        with tc.tile_pool(name="cst", bufs=1) as cst, \
             tc.tile_pool(name="big", bufs=1) as big:

            wp32_s = cst.tile([128, W32_COLS], F32)
            wpr_s = cst.tile([128, WR_COLS], F32R)
            wq_s = wp32_s[:, 0:384]
            wk_s = wp32_s[:, 384:768]
            bq_s = wp32_s[:, 768:771]
            bk_s = wp32_s[:, 771:774]
            id4_s = wp32_s[:, 774:806]
            t2p_s = wp32_s[:, 806:806 + NCH]
            v4n_s = wpr_s[:, 0:4 * NCH]

            t1b_s = big.tile([128, TQ], F32, tag="t1b")
            xk_a = big.tile([128, T // 8], F32, tag="xka")
            xk_b = big.tile([128, T // 8], F32, tag="xkb")
            xq_s = big.tile([128, TQ // 4], F32, tag="xq")
            kt_s = big.tile([128, T // 4], F32, tag="kt")
            qts32 = big.tile([128, TQ // 4], F32, tag="qts32")
            qrep32 = big.tile([128, TQ], F32, tag="qrep32")
            knr = big.tile([128, 32 * NCH], F32R, tag="knr")
            kne = big.tile([128, 32 * NCH], F32R, tag="kne")
            hsball = cst.tile([32, 4 * NT], F32)
            hsbrep = cst.tile([128, 4 * NT], F32)
            zsb = cst.tile([2, TQ], F32)

            nc.sync.dma_start(wp32_s[:], wp32[:])
            nc.sync.dma_start(xq_s[:], xq[:])
            half = T // 8
            nc.scalar.dma_start(xk_a[:], xk[:, 0:half])
            nc.scalar.dma_start(xk_b[:], xk[:, half:])
            nc.sync.dma_start(wpr_s[:], wpr[:])
            nc.gpsimd.dma_start(t1b_s[0:1, :], t1[:])
            p = 1
            while p < 128:
                nc.gpsimd.dma_start(t1b_s[p:2 * p, :], t1b_s[0:p, :])
                p *= 2

            # ---------------- MLPs (block-diag 128-contract) -------------
            with tc.tile_pool(name="mlp", bufs=3, space="PSUM") as mlp, \
                 tc.tile_pool(name="hbuf", bufs=2) as hbuf:

                def run_mlp(x_parts, w_s, b_s, ngrp, dst, dt):
                    h_prev = None
                    for l in range(3):
                        h_next = dst if l == 2 else hbuf.tile(
                            [128, ngrp * 512], dt, tag=f"h{ngrp}",
                            name=f"h{ngrp}_{l}")
                        for G in range(ngrp):
                            if l == 0:
                                npart = len(x_parts)
                                gper = ngrp // npart
                                src_ap = x_parts[G // gper][
                                    :, 512 * (G % gper):512 * (G % gper) + 512]
                            else:
                                src_ap = h_prev[:, 512 * G:512 * G + 512]
                            pt = mlp.tile([128, 512], F32, tag="mlp")
                            nc.tensor.matmul(
                                pt[:], w_s[:, 128 * l:128 * l + 128],
                                src_ap,
                                start=True, stop=True)
                            o = h_next[:, 512 * G:512 * G + 512]
                            if G % 2 == 0:
                                if l < 2:
                                    nc.scalar.activation(
                                        o, pt[:], AF.Relu, bias=b_s[:, l:l + 1])
                                else:
                                    nc.scalar.activation(
                                        o, pt[:], AF.Identity,
                                        bias=b_s[:, l:l + 1])
                            else:
                                if l < 2:
                                    nc.vector.tensor_scalar(
                                        o, pt[:], b_s[:, l:l + 1], 0.0,
                                        OP.add, OP.max)
                                else:
                                    nc.vector.tensor_scalar(
                                        o, pt[:], b_s[:, l:l + 1], None,
                                        OP.add)
                        h_prev = h_next

                run_mlp([xq_s], wq_s, bq_s, 2, qts32, F32)
                run_mlp([xk_a, xk_b], wk_s, bk_s, 4, kt_s, F32)

            # replicated flat Q^T (fp32), issued off-ACT
            for b in range(4):
                for k in range(4):
                    nc.gpsimd.dma_start(
                        qrep32[32 * b:32 * b + 32, :].rearrange(
                            "d (t c) -> d t c", c=128)[:, k::4, :],
                        qts32[32 * k:32 * k + 32, :].rearrange(
                            "d (t c) -> d t c", c=128))

            # knat transposes (fp32) + exact split into knr/kne (fp32r)
            with tc.tile_pool(name="knT", bufs=1, space="PSUM") as knT:
                knrv = knr[:, :].rearrange("p (c d) -> p c d", d=32)
                knev = kne[:, :].rearrange("p (c d) -> p c d", d=32)
                for half in range(2):
                    pts = [knT.tile([128, 256], F32, tag=f"knT{q}",
                                    name=f"knT{q}") for q in range(4)]
                    for i in range(8):
                        for q in range(4):
                            col = 8 * half + i
                            nc.tensor.matmul(
                                pts[q][:, 32 * i:32 * i + 32],
                                kt_s[32 * q:32 * q + 32,
                                     128 * col:128 * col + 128],
                                id4_s[32 * q:32 * q + 32, :],
                                start=True, stop=True,
                                tile_position=(32 * q, 0))
                    for q in range(4):
                        pv = pts[q][:].rearrange("p (c d) -> p c d", d=32)
                        orr = knrv[:, q + 4 * 8 * half::4, :][:, 0:8, :]
                        oe = knev[:, q + 4 * 8 * half::4, :][:, 0:8, :]
                        nc.scalar.activation(orr, pv, AF.Copy)
                        nc.vector.scalar_tensor_tensor(
                            oe, orr.bitcast(F32), -1.0, pv,
                            OP.mult, OP.add)

            # prefix chain (fp32r) + per-tile H snapshots (fp32) -> hsball
            with tc.tile_pool(name="hps", bufs=3, space="PSUM") as hps:
                prev = 0
                for I in range(NT):
                    w = wc[I]
                    dst = hsball[:, 4 * I:4 * I + 4]
                    if w > prev:
                        dps = hps.tile([32, 4], F32, tag="dh")
                        for c in range(prev, w):
                            nc.tensor.matmul(
                                dps[:], knr[:, 32 * c:32 * c + 32],
                                v4n_s[:, 4 * c:4 * c + 4],
                                start=(c == prev), stop=False)
                            nc.tensor.matmul(
                                dps[:, 0:2], kne[:, 32 * c:32 * c + 32],
                                v4n_s[:, 4 * c:4 * c + 2],
                                start=False, stop=(c == w - 1),
                                skip_group_check=True)
                        if I == 0:
                            nc.vector.tensor_copy(dst, dps[:])
                        else:
                            nc.vector.tensor_tensor(
                                dst, hsball[:, 4 * I - 4:4 * I],
                                dps[:], OP.add)
                        prev = w
                    elif I == 0:
                        nc.vector.memset(dst, 0)
                    else:
                        nc.vector.tensor_copy(
                            dst, hsball[:, 4 * I - 4:4 * I])

            # replicate H table (fp32) to all 4 partition blocks
            for q in range(4):
                nc.gpsimd.dma_start(hsbrep[32 * q:32 * q + 32, :], hsball[:])


            # ---------------- band ----------------
            with tc.tile_pool(name="stp", bufs=3, space="PSUM") as stp, \
                 tc.tile_pool(name="zob", bufs=1, space="PSUM") as zob, \
                 tc.tile_pool(name="zop", bufs=1, space="PSUM") as zop, \
                 tc.tile_pool(name="smp", bufs=10) as smp, \
             tc.tile_pool(name="zps", bufs=2) as zpsp:

                # rotated fp32 prefix-zo straight into zoB rows 0:4
                zoBs = {}

                def emit_prefix_batch(gs):
                    for g in gs:
                        zoBs[g] = zob.tile([4, 512], F32, tag=f"zoB{g % 4}",
                                           name=f"zoB{g % 4}")
                    for t in range(4):
                        for g in gs:
                            I = 4 * g + t
                            qq = g % 4
                            nc.tensor.matmul(
                                zoBs[g][:, 128 * t:128 * t + 128],
                                hsbrep[32 * qq:32 * qq + 32, 4 * I:4 * I + 4],
                                qrep32[32 * qq:32 * qq + 32,
                                       128 * I:128 * I + 128],
                                start=(t == 0), stop=False,
                                tile_position=(32 * qq, 0),
                                skip_group_check=True)

                emit_prefix_batch(range(0, 4))
                made = {}
                for g in range(NG):
                    if g == 4:
                        emit_prefix_batch(range(4, 8))
                    zoB = zoBs[g]
                    for (idx, a, b) in gparts[g]:
                        if idx in made:
                            continue
                        c, qlo, lo, ihi = pieces[idx]
                        wd = 128 * ihi - qlo
                        q = c % 4
                        stb = stp.tile([128, 512], F32, tag="st")
                        nc.tensor.matmul(
                            stb[:, 0:wd],
                            kt_s[32 * q:32 * q + 32,
                                 128 * (c // 4):128 * (c // 4) + 128],
                            qrep32[32 * q:32 * q + 32, qlo:128 * ihi],
                            start=True, stop=True,
                            tile_position=(32 * q, 0))
                        smt = smp.tile([128, 512], F32R, tag="smt")
                        nc.vector.scalar_tensor_tensor(
                            smt[:, 0:wd], t1b_s[:, qlo:128 * ihi],
                            t2p_s[:, c:c + 1], stb[:, 0:wd],
                            OP.is_ge, OP.mult)
                        made[idx] = smt
                    nparts = len(gparts[g])
                    assert nparts > 0
                    for i, (idx, a, b) in enumerate(gparts[g]):
                        c, qlo, lo, ihi = pieces[idx]
                        smt = made[idx]
                        nc.tensor.matmul(
                            zoB[:, a - 512 * g:b - 512 * g],
                            v4n_s[:, 4 * c:4 * c + 4],
                            smt[:, a - qlo:b - qlo],
                            start=False, stop=(i == nparts - 1),
                            skip_group_check=True)
                    zb4 = zpsp.tile([4, 512], F32, tag="zb4")
                    nc.scalar.activation(zb4[:], zoB[:], AF.Copy)
                    zbs = zpsp.tile([2, 512], F32, tag="zbs")
                    nc.gpsimd.dma_start(zbs[:], zb4[2:4, :])
                    nc.vector.tensor_tensor(
                        zsb[:, 512 * g:512 * g + 512], zb4[0:2, :], zbs[:],
                        OP.add)

            nc.sync.dma_start(out[:], zsb[:])
            nc.sync.dma_start(hdbg[:], hsball[:])
    nc.finalize()
    return nc


_CACHE = {}
LAST_RESULTS = None


def kernel(m1, m2, m3, m4, Wq, bq, Wk, bk):
    mods = [np.asarray(m)[0, 0].astype(np.float32) for m in (m1, m2, m3, m4)]
    Wq, bq, Wk, bk = (np.asarray(a, dtype=np.float32) for a in (Wq, bq, Wk, bk))
    t2s = [m[:, -1].copy() for m in mods]
    t1g = mods[0][:, -1].copy()

    def qsel(h):
        idx = np.arange(TQ)
        gt = 2 * (idx // 128) + h
        return gt * 128 + (idx % 128)

    sels = [qsel(0), qsel(1)]
    t1_locals = [t1g[s] for s in sels]
    wc, e = _band_meta(t1_locals, t2s)
    pieces = _pieces_meta(wc, e, t1_locals, t2s)

    key = (tuple(wc), tuple(e), tuple(p[1] for p in pieces))
    if key not in _CACHE:
        _CACHE[key] = _build(wc, e, pieces)
    nc = _CACHE[key]

    def chop11(x):
        # zero low 12 mantissa bits: exactly representable in fp32r (11b)
        xm = np.ascontiguousarray(x, np.float32).view(np.uint32)
        return (xm & np.uint32(0xFFFFF000)).view(np.float32).copy()

    def blockdiag(W):
        out = np.zeros((128, 384), dtype=np.float32)
        for l in range(3):
            for k in range(4):
                out[32 * k:32 * k + 32,
                    128 * l + 32 * k:128 * l + 32 * k + 32] = W[l]
        return out

    bq_in = np.tile(bq.T, (4, 1)).astype(np.float32)
    bk_in = np.tile(bk.T, (4, 1)).astype(np.float32)
    id4_in = np.tile(np.eye(32, dtype=np.float32), (4, 1))

    in_maps = []
    for core in range(8):
        mod, h = core // 2, core % 2
        x = mods[mod]
        t2 = t2s[mod]
        xk_in = _s4(np.ascontiguousarray(x.T))
        xq_l = mods[0][sels[h]]
        xq_in = _s4(np.ascontiguousarray(xq_l.T))
        t1_in = t1_locals[h].reshape(1, TQ).astype(np.float32)
        t2p_in = np.ascontiguousarray(t2.reshape(NCH, 128).T)
        v2n_in = np.ascontiguousarray(
            x[:, :2].reshape(NCH, 128, 2).transpose(1, 0, 2)
            .reshape(128, 2 * NCH))
        v2nr_in = chop11(v2n_in)
        v2ne_in = v2n_in - v2nr_in
        v4n_in = np.zeros((128, 4 * NCH), dtype=np.float32)
        v4n_in[:, 0::4] = v2nr_in[:, 0::2]
        v4n_in[:, 1::4] = v2nr_in[:, 1::2]
        v4n_in[:, 2::4] = v2ne_in[:, 0::2]
        v4n_in[:, 3::4] = v2ne_in[:, 1::2]
        wp32_in = np.concatenate(
            [blockdiag(Wq), blockdiag(Wk), bq_in, bk_in, id4_in, t2p_in],
            axis=1)
        wpr_in = v4n_in
        in_maps.append({
            "xk": xk_in, "xq": xq_in, "wp32": wp32_in, "wpr": wpr_in,
            "t1": t1_in,
        })

    import os as _os
    trace = bool(_os.environ.get("KERNEL_TRACE"))
    res = run_bass_kernel_spmd(nc, in_maps, core_ids=list(range(8)),
                               trace=trace)
    global LAST_RESULTS
    LAST_RESULTS = res

    y = np.zeros((T, 2), dtype=np.float32)
    for core in range(8):
        mod, h = core // 2, core % 2
        zt = res.results[core]["out"]
        y[sels[h]] += zt.T
    return y[None, :, :]


# revision 21
# speedup vs baseline: 1.0659x; 1.0077x over previous
"""Trainium2 Bass kernel for sparse_attention problem nn_CAMD_73229192397362.

v4 precision model (HW-validated: fp32r = round-to-nearest ~11-bit, but
the tolerance needs ~15+ bits on every path feeding the 7e4-magnitude
accumulations):
  - Both MLPs, band S^T, knat transposes, H snapshots and prefix-zo run
    in fp32.
  - The prefix chain K^T V runs as THREE fp32r matmuls per chunk
    (Kr Vr + Kr Ve + Ke Vr) with exact splits: V split on host,
    K split on-chip from the fp32 knat (round-copy + subtract).
  - The band zo runs fp32r on the fp32-exact masked S (smt) and Vr; its
    residuals are per-key random and average out over the band.

Structure per core (8 = 4 modalities x 2 query half-sets):
  stacked s4 layout (chunk c -> partitions 32*(c%4), cols 128*(c//4));
  block-diag 128-contract MLPs; rotated (tile_position) 32-contract
  band S^T and knat transposes; per-tile prefix H folded in via rotated
  fp32 prefix-zo into 4 PSUM banks (zoP), combined with the band zo
  accumulator (zoB) on DVE at the end of each 512-query group.
"""

import numpy as np

import concourse.bass as bass
from concourse.bacc import Bacc
import concourse.mybir as mybir
from concourse.tile import TileContext
from concourse.bass_utils import run_bass_kernel_spmd

T = 8192
D = 32
TQ = 4096
NT = TQ // 128
NCH = T // 128
NG = NT // 4
F32 = mybir.dt.float32
F32R = mybir.dt.float32r
AF = mybir.ActivationFunctionType
OP = mybir.AluOpType

# packed input column maps
W32_COLS = 384 + 384 + 3 + 3 + 32 + NCH   # wq | wk | bq | bk | id4 | t2p
WR_COLS = 4 * NCH                         # v4n: per chunk [vr0 vr1 ve0 ve1]


def _s4(xT):
    """(32, N) -> (128, N//4): 128-col chunk c -> partitions 32*(c%4),
    cols 128*(c//4)."""
    d, N = xT.shape
    nch = N // 128
    out = np.zeros((128, N // 4), dtype=xT.dtype)
    for c in range(nch):
        out[32 * (c % 4):32 * (c % 4) + 32,
            128 * (c // 4):128 * (c // 4) + 128] = xT[:, 128 * c:128 * c + 128]
    return out


def _band_meta(t1_all, t2_all):
    w_raw = np.full(NT, T, dtype=np.int64)
    for t1 in t1_all:
        for t2 in t2_all:
            r_min = np.searchsorted(t2, t1[::128], side="right")
            w_raw = np.minimum(w_raw, (r_min // 128) * 128)
    e = np.zeros(NT, dtype=np.int64)
    for t1 in t1_all:
        for t2 in t2_all:
            r_max = np.searchsorted(t2, t1[127::128], side="right")
            e = np.maximum(e, (r_max + 127) // 128)
    wc = w_raw // 128
    e = np.maximum(e, wc + 1)
    e = np.minimum(np.maximum.accumulate(e), NCH)
    wc = np.minimum(wc, e - 1)
    assert np.all(np.diff(wc) >= 0) and np.all(np.diff(e) >= 0)
    for t1 in t1_all:
        for t2 in t2_all:
            r_min = np.searchsorted(t2, t1[::128], side="right")
            r_max = np.searchsorted(t2, t1[127::128], side="right")
            assert np.all(wc * 128 <= r_min) and np.all(r_max <= e * 128)
    return [int(x) for x in wc], [int(x) for x in e]


def _pieces_meta(wc, e, t1_all, t2_all):
    pieces = []
    for c in range(NCH):
        tiles = [I for I in range(NT) if wc[I] <= c < e[I]]
        if not tiles:
            continue
        lo, ihi = tiles[0], tiles[-1] + 1
        qlo = 128 * lo
        qmin = TQ
        for t1 in t1_all:
            for t2 in t2_all:
                qmin = min(qmin, int(np.searchsorted(t1, t2[128 * c])))
        qlo = max(qlo, (qmin // 64) * 64)
        qlo = min(qlo, 128 * ihi - 64)
        pieces.append((c, qlo, lo, ihi))
    return pieces


def _build(wc, e, pieces):
    nc = Bacc("TRN2")

    xk = nc.dram_tensor("xk", [128, T // 4], F32, kind="ExternalInput")
    xq = nc.dram_tensor("xq", [128, TQ // 4], F32, kind="ExternalInput")
    wp32 = nc.dram_tensor("wp32", [128, W32_COLS], F32, kind="ExternalInput")
    wpr = nc.dram_tensor("wpr", [128, WR_COLS], F32R, kind="ExternalInput")
    t1 = nc.dram_tensor("t1", [1, TQ], F32, kind="ExternalInput")
    out = nc.dram_tensor("out", [2, TQ], F32, kind="ExternalOutput")
    hdbg = nc.dram_tensor("hdbg", [32, 4 * NT], F32, kind="ExternalOutput")

    maxw = max(wc)

    gparts = {g: [] for g in range(NG)}
    for idx, (c, qlo, lo, ihi) in enumerate(pieces):
        for g in range((qlo // 512), (ihi * 128 - 1) // 512 + 1):
            a = max(qlo, 512 * g)
            b = min(128 * ihi, 512 * g + 512)
            gparts[g].append((idx, a, b))

    with TileContext(nc) as tc:
        with tc.tile_pool(name="cst", bufs=1) as cst, \
             tc.tile_pool(name="big", bufs=1) as big:

            wp32_s = cst.tile([128, W32_COLS], F32)
            wpr_s = cst.tile([128, WR_COLS], F32R)
            wq_s = wp32_s[:, 0:384]
            wk_s = wp32_s[:, 384:768]
            bq_s = wp32_s[:, 768:771]
            bk_s = wp32_s[:, 771:774]
            id4_s = wp32_s[:, 774:806]
            t2p_s = wp32_s[:, 806:806 + NCH]
            v4n_s = wpr_s[:, 0:4 * NCH]

            t1b_s = big.tile([128, TQ], F32, tag="t1b")
            xk_a = big.tile([128, T // 8], F32, tag="xka")
            xk_b = big.tile([128, T // 8], F32, tag="xkb")
            xq_s = big.tile([128, TQ // 4], F32, tag="xq")
            kt_s = big.tile([128, T // 4], F32, tag="kt")
            qts32 = big.tile([128, TQ // 4], F32, tag="qts32")
            qrep32 = big.tile([128, TQ], F32, tag="qrep32")
            qrepr = big.tile([128, TQ], F32R, tag="qrepr")
            qrepe = big.tile([128, TQ], F32R, tag="qrepe")
            ktr = big.tile([128, T // 4], F32R, tag="ktr")
            knr = big.tile([128, 32 * NCH], F32R, tag="knr")
            kne = big.tile([128, 32 * NCH], F32R, tag="kne")
            hsball = cst.tile([32, 4 * NT], F32)
            hsbrep = cst.tile([128, 4 * NT], F32)
            zsb = cst.tile([2, TQ], F32)

            nc.sync.dma_start(wp32_s[:], wp32[:])
            nc.sync.dma_start(xq_s[:], xq[:])
            half = T // 8
            nc.scalar.dma_start(xk_a[:], xk[:, 0:half])
            nc.scalar.dma_start(xk_b[:], xk[:, half:])
            nc.sync.dma_start(wpr_s[:], wpr[:])
            nc.gpsimd.dma_start(t1b_s[0:1, :], t1[:])
            p = 1
            while p < 128:
                nc.gpsimd.dma_start(t1b_s[p:2 * p, :], t1b_s[0:p, :])
                p *= 2

            # ---------------- MLPs (block-diag 128-contract) -------------
            with tc.tile_pool(name="mlp", bufs=3, space="PSUM") as mlp, \
                 tc.tile_pool(name="hbuf", bufs=2) as hbuf:

                def run_mlp(x_parts, w_s, b_s, ngrp, dst, dt):
                    h_prev = None
                    for l in range(3):
                        h_next = dst if l == 2 else hbuf.tile(
                            [128, ngrp * 512], dt, tag=f"h{ngrp}",
                            name=f"h{ngrp}_{l}")
                        for G in range(ngrp):
                            if l == 0:
                                npart = len(x_parts)
                                gper = ngrp // npart
                                src_ap = x_parts[G // gper][
                                    :, 512 * (G % gper):512 * (G % gper) + 512]
                            else:
                                src_ap = h_prev[:, 512 * G:512 * G + 512]
                            pt = mlp.tile([128, 512], F32, tag="mlp")
                            nc.tensor.matmul(
                                pt[:], w_s[:, 128 * l:128 * l + 128],
                                src_ap,
                                start=True, stop=True)
                            o = h_next[:, 512 * G:512 * G + 512]
                            if G % 2 == 0:
                                if l < 2:
                                    nc.scalar.activation(
                                        o, pt[:], AF.Relu, bias=b_s[:, l:l + 1])
                                else:
                                    nc.scalar.activation(
                                        o, pt[:], AF.Identity,
                                        bias=b_s[:, l:l + 1])
                            else:
                                if l < 2:
                                    nc.vector.tensor_scalar(
                                        o, pt[:], b_s[:, l:l + 1], 0.0,
                                        OP.add, OP.max)
                                else:
                                    nc.vector.tensor_scalar(
                                        o, pt[:], b_s[:, l:l + 1], None,
                                        OP.add)
                        h_prev = h_next

                run_mlp([xq_s], wq_s, bq_s, 2, qts32, F32)
                run_mlp([xk_a, xk_b], wk_s, bk_s, 4, kt_s, F32)

            # replicated flat Q^T (fp32), issued off-ACT
            for b in range(4):
                for k in range(4):
                    nc.gpsimd.dma_start(
                        qrep32[32 * b:32 * b + 32, :].rearrange(
                            "d (t c) -> d t c", c=128)[:, k::4, :],
                        qts32[32 * k:32 * k + 32, :].rearrange(
                            "d (t c) -> d t c", c=128))

            # round/residual copies for the fp32r band path
            for j in range(4):
                sl = slice(512 * j, 512 * j + 512)
                nc.scalar.activation(ktr[:, sl], kt_s[:, sl], AF.Copy)
            for j in range(8):
                sl = slice(512 * j, 512 * j + 512)
                if j % 2 == 0:
                    nc.scalar.activation(qrepr[:, sl], qrep32[:, sl], AF.Copy)
                else:
                    nc.vector.tensor_copy(qrepr[:, sl], qrep32[:, sl])
                nc.vector.scalar_tensor_tensor(
                    qrepe[:, sl], qrepr[:, sl].bitcast(F32), -1.0,
                    qrep32[:, sl], OP.mult, OP.add)

            # knat transposes (fp32) + exact split into knr/kne (fp32r)
            with tc.tile_pool(name="knT", bufs=1, space="PSUM") as knT:
                knrv = knr[:, :].rearrange("p (c d) -> p c d", d=32)
                knev = kne[:, :].rearrange("p (c d) -> p c d", d=32)
                for half in range(2):
                    pts = [knT.tile([128, 256], F32, tag=f"knT{q}",
                                    name=f"knT{q}") for q in range(4)]
                    for i in range(8):
                        for q in range(4):
                            col = 8 * half + i
                            nc.tensor.matmul(
                                pts[q][:, 32 * i:32 * i + 32],
                                kt_s[32 * q:32 * q + 32,
                                     128 * col:128 * col + 128],
                                id4_s[32 * q:32 * q + 32, :],
                                start=True, stop=True,
                                tile_position=(32 * q, 0))
                    for q in range(4):
                        pv = pts[q][:].rearrange("p (c d) -> p c d", d=32)
                        orr = knrv[:, q + 4 * 8 * half::4, :][:, 0:8, :]
                        oe = knev[:, q + 4 * 8 * half::4, :][:, 0:8, :]
                        nc.scalar.activation(orr, pv, AF.Copy)
                        nc.vector.scalar_tensor_tensor(
                            oe, orr.bitcast(F32), -1.0, pv,
                            OP.mult, OP.add)

            # prefix chain (fp32r) + per-tile H snapshots (fp32) -> hsball
            with tc.tile_pool(name="hps", bufs=3, space="PSUM") as hps:
                prev = 0
                for I in range(NT):
                    w = wc[I]
                    dst = hsball[:, 4 * I:4 * I + 4]
                    if w > prev:
                        dps = hps.tile([32, 4], F32, tag="dh")
                        for c in range(prev, w):
                            nc.tensor.matmul(
                                dps[:], knr[:, 32 * c:32 * c + 32],
                                v4n_s[:, 4 * c:4 * c + 4],
                                start=(c == prev), stop=False)
                            nc.tensor.matmul(
                                dps[:, 0:2], kne[:, 32 * c:32 * c + 32],
                                v4n_s[:, 4 * c:4 * c + 2],
                                start=False, stop=(c == w - 1),
                                skip_group_check=True)
                        if I == 0:
                            nc.vector.tensor_copy(dst, dps[:])
                        else:
                            nc.vector.tensor_tensor(
                                dst, hsball[:, 4 * I - 4:4 * I],
                                dps[:], OP.add)
                        prev = w
                    elif I == 0:
                        nc.vector.memset(dst, 0)
                    else:
                        nc.vector.tensor_copy(
                            dst, hsball[:, 4 * I - 4:4 * I])

            # replicate H table (fp32) to all 4 partition blocks
            for q in range(4):
                nc.gpsimd.dma_start(hsbrep[32 * q:32 * q + 32, :], hsball[:])


            # ---------------- band ----------------
            with tc.tile_pool(name="stp", bufs=3, space="PSUM") as stp, \
                 tc.tile_pool(name="zob", bufs=1, space="PSUM") as zob, \
                 tc.tile_pool(name="zop", bufs=1, space="PSUM") as zop, \
                 tc.tile_pool(name="smp", bufs=10) as smp, \
             tc.tile_pool(name="zps", bufs=2) as zpsp:

                # rotated fp32 prefix-zo straight into zoB rows 0:4
                zoBs = {}

                def emit_prefix_batch(gs):
                    for g in gs:
                        zoBs[g] = zob.tile([4, 512], F32, tag=f"zoB{g % 4}",
                                           name=f"zoB{g % 4}")
                    for t in range(4):
                        for g in gs:
                            I = 4 * g + t
                            qq = g % 4
                            nc.tensor.matmul(
                                zoBs[g][:, 128 * t:128 * t + 128],
                                hsbrep[32 * qq:32 * qq + 32, 4 * I:4 * I + 4],
                                qrep32[32 * qq:32 * qq + 32,
                                       128 * I:128 * I + 128],
                                start=(t == 0), stop=False,
                                tile_position=(32 * qq, 0),
                                skip_group_check=True)

                emit_prefix_batch(range(0, 4))
                made = {}
                for g in range(NG):
                    if g == 4:
                        emit_prefix_batch(range(4, 8))
                    zoB = zoBs[g]
                    for (idx, a, b) in gparts[g]:
                        if idx in made:
                            continue
                        c, qlo, lo, ihi = pieces[idx]
                        wd = 128 * ihi - qlo
                        q = c % 4
                        stb = stp.tile([128, 512], F32, tag="st")
                        kslice = ktr[32 * q:32 * q + 32,
                                     128 * (c // 4):128 * (c // 4) + 128]
                        nc.tensor.matmul(
                            stb[:, 0:wd], kslice,
                            qrepr[32 * q:32 * q + 32, qlo:128 * ihi],
                            start=True, stop=False,
                            tile_position=(32 * q, 0))
                        nc.tensor.matmul(
                            stb[:, 0:wd], kslice,
                            qrepe[32 * q:32 * q + 32, qlo:128 * ihi],
                            start=False, stop=True,
                            tile_position=(32 * q, 0),
                            skip_group_check=True)
                        smt = smp.tile([128, 512], F32R, tag="smt")
                        nc.vector.scalar_tensor_tensor(
                            smt[:, 0:wd], t1b_s[:, qlo:128 * ihi],
                            t2p_s[:, c:c + 1], stb[:, 0:wd],
                            OP.is_ge, OP.mult)
                        made[idx] = smt
                    nparts = len(gparts[g])
                    assert nparts > 0
                    for i, (idx, a, b) in enumerate(gparts[g]):
                        c, qlo, lo, ihi = pieces[idx]
                        smt = made[idx]
                        nc.tensor.matmul(
                            zoB[:, a - 512 * g:b - 512 * g],
                            v4n_s[:, 4 * c:4 * c + 4],
                            smt[:, a - qlo:b - qlo],
                            start=False, stop=(i == nparts - 1),
                            skip_group_check=True)
                    zb4 = zpsp.tile([4, 512], F32, tag="zb4")
                    nc.scalar.activation(zb4[:], zoB[:], AF.Copy)
                    zbs = zpsp.tile([2, 512], F32, tag="zbs")
                    nc.gpsimd.dma_start(zbs[:], zb4[2:4, :])
                    nc.vector.tensor_tensor(
                        zsb[:, 512 * g:512 * g + 512], zb4[0:2, :], zbs[:],
                        OP.add)

            nc.sync.dma_start(out[:], zsb[:])
            nc.sync.dma_start(hdbg[:], hsball[:])
    nc.finalize()
    return nc


_CACHE = {}
LAST_RESULTS = None


def kernel(m1, m2, m3, m4, Wq, bq, Wk, bk):
    mods = [np.asarray(m)[0, 0].astype(np.float32) for m in (m1, m2, m3, m4)]
    Wq, bq, Wk, bk = (np.asarray(a, dtype=np.float32) for a in (Wq, bq, Wk, bk))
    t2s = [m[:, -1].copy() for m in mods]
    t1g = mods[0][:, -1].copy()

    def qsel(h):
        idx = np.arange(TQ)
        gt = 2 * (idx // 128) + h
        return gt * 128 + (idx % 128)

    sels = [qsel(0), qsel(1)]
    t1_locals = [t1g[s] for s in sels]
    wc, e = _band_meta(t1_locals, t2s)
    pieces = _pieces_meta(wc, e, t1_locals, t2s)

    key = (tuple(wc), tuple(e), tuple(p[1] for p in pieces))
    if key not in _CACHE:
        _CACHE[key] = _build(wc, e, pieces)
    nc = _CACHE[key]

    def chop11(x):
        # zero low 12 mantissa bits: exactly representable in fp32r (11b)
        xm = np.ascontiguousarray(x, np.float32).view(np.uint32)
        return (xm & np.uint32(0xFFFFF000)).view(np.float32).copy()

    def blockdiag(W):
        out = np.zeros((128, 384), dtype=np.float32)
        for l in range(3):
            for k in range(4):
                out[32 * k:32 * k + 32,
                    128 * l + 32 * k:128 * l + 32 * k + 32] = W[l]
        return out

    bq_in = np.tile(bq.T, (4, 1)).astype(np.float32)
    bk_in = np.tile(bk.T, (4, 1)).astype(np.float32)
    id4_in = np.tile(np.eye(32, dtype=np.float32), (4, 1))

    in_maps = []
    for core in range(8):
        mod, h = core // 2, core % 2
        x = mods[mod]
        t2 = t2s[mod]
        xk_in = _s4(np.ascontiguousarray(x.T))
        xq_l = mods[0][sels[h]]
        xq_in = _s4(np.ascontiguousarray(xq_l.T))
        t1_in = t1_locals[h].reshape(1, TQ).astype(np.float32)
        t2p_in = np.ascontiguousarray(t2.reshape(NCH, 128).T)
        v2n_in = np.ascontiguousarray(
            x[:, :2].reshape(NCH, 128, 2).transpose(1, 0, 2)
            .reshape(128, 2 * NCH))
        v2nr_in = chop11(v2n_in)
        v2ne_in = v2n_in - v2nr_in
        v4n_in = np.zeros((128, 4 * NCH), dtype=np.float32)
        v4n_in[:, 0::4] = v2nr_in[:, 0::2]
        v4n_in[:, 1::4] = v2nr_in[:, 1::2]
        v4n_in[:, 2::4] = v2ne_in[:, 0::2]
        v4n_in[:, 3::4] = v2ne_in[:, 1::2]
        wp32_in = np.concatenate(
            [blockdiag(Wq), blockdiag(Wk), bq_in, bk_in, id4_in, t2p_in],
            axis=1)
        wpr_in = v4n_in
        in_maps.append({
            "xk": xk_in, "xq": xq_in, "wp32": wp32_in, "wpr": wpr_in,
            "t1": t1_in,
        })

    import os as _os
    trace = bool(_os.environ.get("KERNEL_TRACE"))
    res = run_bass_kernel_spmd(nc, in_maps, core_ids=list(range(8)),
                               trace=trace)
    global LAST_RESULTS
    LAST_RESULTS = res

    y = np.zeros((T, 2), dtype=np.float32)
    for core in range(8):
        mod, h = core // 2, core % 2
        zt = res.results[core]["out"]
        y[sels[h]] += zt.T
    return y[None, :, :]


# revision 22
# speedup vs baseline: 1.1454x; 1.0746x over previous
"""Trainium2 Bass kernel for sparse_attention problem nn_CAMD_73229192397362.

v4 precision model (HW-validated: fp32r = round-to-nearest ~11-bit, but
the tolerance needs ~15+ bits on every path feeding the 7e4-magnitude
accumulations):
  - Both MLPs, band S^T, knat transposes, H snapshots and prefix-zo run
    in fp32.
  - The prefix chain K^T V runs as THREE fp32r matmuls per chunk
    (Kr Vr + Kr Ve + Ke Vr) with exact splits: V split on host,
    K split on-chip from the fp32 knat (round-copy + subtract).
  - The band zo runs fp32r on the fp32-exact masked S (smt) and Vr; its
    residuals are per-key random and average out over the band.

Structure per core (8 = 4 modalities x 2 query half-sets):
  stacked s4 layout (chunk c -> partitions 32*(c%4), cols 128*(c//4));
  block-diag 128-contract MLPs; rotated (tile_position) 32-contract
  band S^T and knat transposes; per-tile prefix H folded in via rotated
  fp32 prefix-zo into 4 PSUM banks (zoP), combined with the band zo
  accumulator (zoB) on DVE at the end of each 512-query group.
"""

import numpy as np

import concourse.bass as bass
from concourse.bacc import Bacc
import concourse.mybir as mybir
from concourse.tile import TileContext
from concourse.bass_utils import run_bass_kernel_spmd

T = 8192
D = 32
TQ = 4096
NT = TQ // 128
NCH = T // 128
NG = NT // 4
F32 = mybir.dt.float32
F32R = mybir.dt.float32r
AF = mybir.ActivationFunctionType
OP = mybir.AluOpType

# packed input column maps
W32_COLS = 384 + 384 + 3 + 3 + 32 + NCH   # wq | wk | bq | bk | id4 | t2p
WR_COLS = 4 * NCH                         # v4n: per chunk [vr0 vr1 ve0 ve1]


def _s4(xT):
    """(32, N) -> (128, N//4): 128-col chunk c -> partitions 32*(c%4),
    cols 128*(c//4)."""
    d, N = xT.shape
    nch = N // 128
    out = np.zeros((128, N // 4), dtype=xT.dtype)
    for c in range(nch):
        out[32 * (c % 4):32 * (c % 4) + 32,
            128 * (c // 4):128 * (c // 4) + 128] = xT[:, 128 * c:128 * c + 128]
    return out


def _band_meta(t1_all, t2_all):
    w_raw = np.full(NT, T, dtype=np.int64)
    for t1 in t1_all:
        for t2 in t2_all:
            r_min = np.searchsorted(t2, t1[::128], side="right")
            w_raw = np.minimum(w_raw, (r_min // 128) * 128)
    e = np.zeros(NT, dtype=np.int64)
    for t1 in t1_all:
        for t2 in t2_all:
            r_max = np.searchsorted(t2, t1[127::128], side="right")
            e = np.maximum(e, (r_max + 127) // 128)
    wc = w_raw // 128
    e = np.maximum(e, wc + 1)
    e = np.minimum(np.maximum.accumulate(e), NCH)
    wc = np.minimum(wc, e - 1)
    assert np.all(np.diff(wc) >= 0) and np.all(np.diff(e) >= 0)
    for t1 in t1_all:
        for t2 in t2_all:
            r_min = np.searchsorted(t2, t1[::128], side="right")
            r_max = np.searchsorted(t2, t1[127::128], side="right")
            assert np.all(wc * 128 <= r_min) and np.all(r_max <= e * 128)
    return [int(x) for x in wc], [int(x) for x in e]


def _pieces_meta(wc, e, t1_all, t2_all):
    pieces = []
    for c in range(NCH):
        tiles = [I for I in range(NT) if wc[I] <= c < e[I]]
        if not tiles:
            continue
        lo, ihi = tiles[0], tiles[-1] + 1
        qlo = 128 * lo
        qmin = TQ
        for t1 in t1_all:
            for t2 in t2_all:
                qmin = min(qmin, int(np.searchsorted(t1, t2[128 * c])))
        qlo = max(qlo, (qmin // 64) * 64)
        qlo = min(qlo, 128 * ihi - 64)
        pieces.append((c, qlo, lo, ihi))
    return pieces


def _build(wc, e, pieces):
    nc = Bacc("TRN2")

    xk = nc.dram_tensor("xk", [128, T // 4], F32, kind="ExternalInput")
    xq = nc.dram_tensor("xq", [128, TQ // 4], F32, kind="ExternalInput")
    wp32 = nc.dram_tensor("wp32", [128, W32_COLS], F32, kind="ExternalInput")
    wpr = nc.dram_tensor("wpr", [128, WR_COLS], F32R, kind="ExternalInput")
    t1 = nc.dram_tensor("t1", [1, TQ], F32, kind="ExternalInput")
    out = nc.dram_tensor("out", [2, TQ], F32, kind="ExternalOutput")
    hdbg = nc.dram_tensor("hdbg", [32, 4 * NT], F32, kind="ExternalOutput")

    maxw = max(wc)

    gparts = {g: [] for g in range(NG)}
    for idx, (c, qlo, lo, ihi) in enumerate(pieces):
        for g in range((qlo // 512), (ihi * 128 - 1) // 512 + 1):
            a = max(qlo, 512 * g)
            b = min(128 * ihi, 512 * g + 512)
            gparts[g].append((idx, a, b))

    with TileContext(nc) as tc:
        with tc.tile_pool(name="cst", bufs=1) as cst, \
             tc.tile_pool(name="big", bufs=1) as big:

            wp32_s = cst.tile([128, W32_COLS], F32)
            wpr_s = cst.tile([128, WR_COLS], F32R)
            wq_s = wp32_s[:, 0:384]
            wk_s = wp32_s[:, 384:768]
            bq_s = wp32_s[:, 768:771]
            bk_s = wp32_s[:, 771:774]
            id4_s = wp32_s[:, 774:806]
            t2p_s = wp32_s[:, 806:806 + NCH]
            v4n_s = wpr_s[:, 0:4 * NCH]

            t1b_s = big.tile([128, TQ], F32, tag="t1b")
            xk_a = big.tile([128, T // 8], F32, tag="xka")
            xk_b = big.tile([128, T // 8], F32, tag="xkb")
            xq_s = big.tile([128, TQ // 4], F32, tag="xq")
            kt_s = big.tile([128, T // 4], F32, tag="kt")
            qts32 = big.tile([128, TQ // 4], F32, tag="qts32")
            qrep32 = big.tile([128, TQ], F32, tag="qrep32")
            qrepr = big.tile([128, TQ], F32R, tag="qrepr")
            qrepe = big.tile([128, TQ], F32R, tag="qrepe")
            ktr = big.tile([128, T // 4], F32R, tag="ktr")
            knr = big.tile([128, 32 * NCH], F32R, tag="knr")
            kne = big.tile([128, 32 * NCH], F32R, tag="kne")
            hsball = cst.tile([32, 4 * NT], F32)
            hsbrep = cst.tile([128, 4 * NT], F32)
            zsb = cst.tile([2, TQ], F32)

            nc.sync.dma_start(wp32_s[:], wp32[:])
            nc.sync.dma_start(xq_s[:], xq[:])
            half = T // 8
            nc.scalar.dma_start(xk_a[:], xk[:, 0:half])
            nc.scalar.dma_start(xk_b[:], xk[:, half:])
            nc.sync.dma_start(wpr_s[:], wpr[:])
            nc.gpsimd.dma_start(t1b_s[0:1, :], t1[:])
            p = 1
            while p < 128:
                nc.gpsimd.dma_start(t1b_s[p:2 * p, :], t1b_s[0:p, :])
                p *= 2

            # ---------------- MLPs (block-diag 128-contract) -------------
            with tc.tile_pool(name="mlp", bufs=3, space="PSUM") as mlp, \
                 tc.tile_pool(name="hbuf", bufs=2) as hbuf:

                def run_mlp(x_parts, w_s, b_s, ngrp, dst, dt):
                    h_prev = None
                    for l in range(3):
                        h_next = dst if l == 2 else hbuf.tile(
                            [128, ngrp * 512], dt, tag=f"h{ngrp}",
                            name=f"h{ngrp}_{l}")
                        for G in range(ngrp):
                            if l == 0:
                                npart = len(x_parts)
                                gper = ngrp // npart
                                src_ap = x_parts[G // gper][
                                    :, 512 * (G % gper):512 * (G % gper) + 512]
                            else:
                                src_ap = h_prev[:, 512 * G:512 * G + 512]
                            pt = mlp.tile([128, 512], F32, tag="mlp")
                            nc.tensor.matmul(
                                pt[:], w_s[:, 128 * l:128 * l + 128],
                                src_ap,
                                start=True, stop=True)
                            o = h_next[:, 512 * G:512 * G + 512]
                            if G % 2 == 0:
                                if l < 2:
                                    nc.scalar.activation(
                                        o, pt[:], AF.Relu, bias=b_s[:, l:l + 1])
                                else:
                                    nc.scalar.activation(
                                        o, pt[:], AF.Identity,
                                        bias=b_s[:, l:l + 1])
                            else:
                                if l < 2:
                                    nc.vector.tensor_scalar(
                                        o, pt[:], b_s[:, l:l + 1], 0.0,
                                        OP.add, OP.max)
                                else:
                                    nc.vector.tensor_scalar(
                                        o, pt[:], b_s[:, l:l + 1], None,
                                        OP.add)
                        h_prev = h_next

                run_mlp([xq_s], wq_s, bq_s, 2, qts32, F32)
                run_mlp([xk_a, xk_b], wk_s, bk_s, 4, kt_s, F32)

            # replicated flat Q^T (fp32), issued off-ACT
            for b in range(4):
                for k in range(4):
                    nc.gpsimd.dma_start(
                        qrep32[32 * b:32 * b + 32, :].rearrange(
                            "d (t c) -> d t c", c=128)[:, k::4, :],
                        qts32[32 * k:32 * k + 32, :].rearrange(
                            "d (t c) -> d t c", c=128))

            # round/residual copies for the fp32r band path
            for j in range(4):
                sl = slice(512 * j, 512 * j + 512)
                nc.scalar.activation(ktr[:, sl], kt_s[:, sl], AF.Copy)
            for j in range(8):
                sl = slice(512 * j, 512 * j + 512)
                if j % 2 == 0:
                    nc.scalar.activation(qrepr[:, sl], qrep32[:, sl], AF.Copy)
                else:
                    nc.vector.tensor_copy(qrepr[:, sl], qrep32[:, sl])
                nc.gpsimd.tensor_tensor(
                    qrepe[:, sl], qrep32[:, sl],
                    qrepr[:, sl].bitcast(F32), OP.subtract)

            # knat transposes (fp32) + exact split into knr/kne (fp32r)
            with tc.tile_pool(name="knT", bufs=1, space="PSUM") as knT:
                knrv = knr[:, :].rearrange("p (c d) -> p c d", d=32)
                knev = kne[:, :].rearrange("p (c d) -> p c d", d=32)
                for half in range(2):
                    pts = [knT.tile([128, 256], F32, tag=f"knT{q}",
                                    name=f"knT{q}") for q in range(4)]
                    for i in range(8):
                        for q in range(4):
                            col = 8 * half + i
                            nc.tensor.matmul(
                                pts[q][:, 32 * i:32 * i + 32],
                                kt_s[32 * q:32 * q + 32,
                                     128 * col:128 * col + 128],
                                id4_s[32 * q:32 * q + 32, :],
                                start=True, stop=True,
                                tile_position=(32 * q, 0))
                    for q in range(4):
                        pv = pts[q][:].rearrange("p (c d) -> p c d", d=32)
                        orr = knrv[:, q + 4 * 8 * half::4, :][:, 0:8, :]
                        oe = knev[:, q + 4 * 8 * half::4, :][:, 0:8, :]
                        nc.scalar.activation(orr, pv, AF.Copy)
                        nc.vector.scalar_tensor_tensor(
                            oe, orr.bitcast(F32), -1.0, pv,
                            OP.mult, OP.add)

            # prefix chain (fp32r) + per-tile H snapshots (fp32) -> hsball
            with tc.tile_pool(name="hps", bufs=3, space="PSUM") as hps:
                prev = 0
                for I in range(NT):
                    w = wc[I]
                    dst = hsball[:, 4 * I:4 * I + 4]
                    if w > prev:
                        dps = hps.tile([32, 4], F32, tag="dh")
                        for c in range(prev, w):
                            nc.tensor.matmul(
                                dps[:], knr[:, 32 * c:32 * c + 32],
                                v4n_s[:, 4 * c:4 * c + 4],
                                start=(c == prev), stop=False)
                            nc.tensor.matmul(
                                dps[:, 0:2], kne[:, 32 * c:32 * c + 32],
                                v4n_s[:, 4 * c:4 * c + 2],
                                start=False, stop=(c == w - 1),
                                skip_group_check=True)
                        if I == 0:
                            nc.vector.tensor_copy(dst, dps[:])
                        else:
                            nc.vector.tensor_tensor(
                                dst, hsball[:, 4 * I - 4:4 * I],
                                dps[:], OP.add)
                        prev = w
                    elif I == 0:
                        nc.vector.memset(dst, 0)
                    else:
                        nc.vector.tensor_copy(
                            dst, hsball[:, 4 * I - 4:4 * I])

            # replicate H table (fp32) to all 4 partition blocks
            for q in range(4):
                nc.gpsimd.dma_start(hsbrep[32 * q:32 * q + 32, :], hsball[:])


            # ---------------- band ----------------
            with tc.tile_pool(name="stp", bufs=4, space="PSUM") as stp, \
                 tc.tile_pool(name="zob", bufs=1, space="PSUM") as zob, \
                 tc.tile_pool(name="zop", bufs=1, space="PSUM") as zop, \
                 tc.tile_pool(name="smp", bufs=12) as smp, \
             tc.tile_pool(name="zps", bufs=2) as zpsp:

                # rotated fp32 prefix-zo straight into zoB rows 0:4
                zoBs = {}

                def emit_prefix_batch(gs):
                    for g in gs:
                        zoBs[g] = zob.tile([4, 512], F32, tag=f"zoB{g % 4}",
                                           name=f"zoB{g % 4}")
                    for t in range(4):
                        for g in gs:
                            I = 4 * g + t
                            qq = g % 4
                            nc.tensor.matmul(
                                zoBs[g][:, 128 * t:128 * t + 128],
                                hsbrep[32 * qq:32 * qq + 32, 4 * I:4 * I + 4],
                                qrep32[32 * qq:32 * qq + 32,
                                       128 * I:128 * I + 128],
                                start=(t == 0), stop=False,
                                tile_position=(32 * qq, 0),
                                skip_group_check=True)

                emit_prefix_batch(range(0, 4))
                made = {}
                for g in range(NG):
                    if g == 4:
                        emit_prefix_batch(range(4, 8))
                    zoB = zoBs[g]
                    for (idx, a, b) in gparts[g]:
                        if idx in made:
                            continue
                        c, qlo, lo, ihi = pieces[idx]
                        wd = 128 * ihi - qlo
                        q = c % 4
                        stb = stp.tile([128, 512], F32, tag="st")
                        kslice = ktr[32 * q:32 * q + 32,
                                     128 * (c // 4):128 * (c // 4) + 128]
                        nc.tensor.matmul(
                            stb[:, 0:wd], kslice,
                            qrepr[32 * q:32 * q + 32, qlo:128 * ihi],
                            start=True, stop=False,
                            tile_position=(32 * q, 0))
                        nc.tensor.matmul(
                            stb[:, 0:wd], kslice,
                            qrepe[32 * q:32 * q + 32, qlo:128 * ihi],
                            start=False, stop=True,
                            tile_position=(32 * q, 0),
                            skip_group_check=True)
                        smt = smp.tile([128, 512], F32R, tag="smt")
                        nc.vector.scalar_tensor_tensor(
                            smt[:, 0:wd], t1b_s[:, qlo:128 * ihi],
                            t2p_s[:, c:c + 1], stb[:, 0:wd],
                            OP.is_ge, OP.mult)
                        made[idx] = smt
                    nparts = len(gparts[g])
                    assert nparts > 0
                    for i, (idx, a, b) in enumerate(gparts[g]):
                        c, qlo, lo, ihi = pieces[idx]
                        smt = made[idx]
                        nc.tensor.matmul(
                            zoB[:, a - 512 * g:b - 512 * g],
                            v4n_s[:, 4 * c:4 * c + 4],
                            smt[:, a - qlo:b - qlo],
                            start=False, stop=(i == nparts - 1),
                            skip_group_check=True)
                    zb4 = zpsp.tile([4, 512], F32, tag="zb4")
                    nc.scalar.activation(zb4[:], zoB[:], AF.Copy)
                    zbs = zpsp.tile([2, 512], F32, tag="zbs")
                    nc.gpsimd.dma_start(zbs[:], zb4[2:4, :])
                    nc.gpsimd.tensor_tensor(
                        zsb[:, 512 * g:512 * g + 512], zb4[0:2, :], zbs[:],
                        OP.add)

            nc.sync.dma_start(out[:], zsb[:])
            nc.sync.dma_start(hdbg[:], hsball[:])
    nc.finalize()
    return nc


_CACHE = {}
LAST_RESULTS = None


def kernel(m1, m2, m3, m4, Wq, bq, Wk, bk):
    mods = [np.asarray(m)[0, 0].astype(np.float32) for m in (m1, m2, m3, m4)]
    Wq, bq, Wk, bk = (np.asarray(a, dtype=np.float32) for a in (Wq, bq, Wk, bk))
    t2s = [m[:, -1].copy() for m in mods]
    t1g = mods[0][:, -1].copy()

    def qsel(h):
        idx = np.arange(TQ)
        gt = 2 * (idx // 128) + h
        return gt * 128 + (idx % 128)

    sels = [qsel(0), qsel(1)]
    t1_locals = [t1g[s] for s in sels]
    wc, e = _band_meta(t1_locals, t2s)
    pieces = _pieces_meta(wc, e, t1_locals, t2s)

    key = (tuple(wc), tuple(e), tuple(p[1] for p in pieces))
    if key not in _CACHE:
        _CACHE[key] = _build(wc, e, pieces)
    nc = _CACHE[key]

    def chop11(x):
        # zero low 12 mantissa bits: exactly representable in fp32r (11b)
        xm = np.ascontiguousarray(x, np.float32).view(np.uint32)
        return (xm & np.uint32(0xFFFFF000)).view(np.float32).copy()

    def blockdiag(W):
        out = np.zeros((128, 384), dtype=np.float32)
        for l in range(3):
            for k in range(4):
                out[32 * k:32 * k + 32,
                    128 * l + 32 * k:128 * l + 32 * k + 32] = W[l]
        return out

    bq_in = np.tile(bq.T, (4, 1)).astype(np.float32)
    bk_in = np.tile(bk.T, (4, 1)).astype(np.float32)
    id4_in = np.tile(np.eye(32, dtype=np.float32), (4, 1))

    in_maps = []
    for core in range(8):
        mod, h = core // 2, core % 2
        x = mods[mod]
        t2 = t2s[mod]
        xk_in = _s4(np.ascontiguousarray(x.T))
        xq_l = mods[0][sels[h]]
        xq_in = _s4(np.ascontiguousarray(xq_l.T))
        t1_in = t1_locals[h].reshape(1, TQ).astype(np.float32)
        t2p_in = np.ascontiguousarray(t2.reshape(NCH, 128).T)
        v2n_in = np.ascontiguousarray(
            x[:, :2].reshape(NCH, 128, 2).transpose(1, 0, 2)
            .reshape(128, 2 * NCH))
        v2nr_in = chop11(v2n_in)
        v2ne_in = v2n_in - v2nr_in
        v4n_in = np.zeros((128, 4 * NCH), dtype=np.float32)
        v4n_in[:, 0::4] = v2nr_in[:, 0::2]
        v4n_in[:, 1::4] = v2nr_in[:, 1::2]
        v4n_in[:, 2::4] = v2ne_in[:, 0::2]
        v4n_in[:, 3::4] = v2ne_in[:, 1::2]
        wp32_in = np.concatenate(
            [blockdiag(Wq), blockdiag(Wk), bq_in, bk_in, id4_in, t2p_in],
            axis=1)
        wpr_in = v4n_in
        in_maps.append({
            "xk": xk_in, "xq": xq_in, "wp32": wp32_in, "wpr": wpr_in,
            "t1": t1_in,
        })

    import os as _os
    trace = bool(_os.environ.get("KERNEL_TRACE"))
    res = run_bass_kernel_spmd(nc, in_maps, core_ids=list(range(8)),
                               trace=trace)
    global LAST_RESULTS
    LAST_RESULTS = res

    y = np.zeros((T, 2), dtype=np.float32)
    for core in range(8):
        mod, h = core // 2, core % 2
        zt = res.results[core]["out"]
        y[sels[h]] += zt.T
    return y[None, :, :]


# revision 25
# speedup vs baseline: 1.1694x; 1.0209x over previous
"""Trainium2 Bass kernel for sparse_attention problem nn_CAMD_73229192397362.

v4 precision model (HW-validated: fp32r = round-to-nearest ~11-bit, but
the tolerance needs ~15+ bits on every path feeding the 7e4-magnitude
accumulations):
  - Both MLPs, band S^T, knat transposes, H snapshots and prefix-zo run
    in fp32.
  - The prefix chain K^T V runs as THREE fp32r matmuls per chunk
    (Kr Vr + Kr Ve + Ke Vr) with exact splits: V split on host,
    K split on-chip from the fp32 knat (round-copy + subtract).
  - The band zo runs fp32r on the fp32-exact masked S (smt) and Vr; its
    residuals are per-key random and average out over the band.

Structure per core (8 = 4 modalities x 2 query half-sets):
  stacked s4 layout (chunk c -> partitions 32*(c%4), cols 128*(c//4));
  block-diag 128-contract MLPs; rotated (tile_position) 32-contract
  band S^T and knat transposes; per-tile prefix H folded in via rotated
  fp32 prefix-zo into 4 PSUM banks (zoP), combined with the band zo
  accumulator (zoB) on DVE at the end of each 512-query group.
"""

import numpy as np

import concourse.bass as bass
from concourse.bacc import Bacc
import concourse.mybir as mybir
from concourse.tile import TileContext
from concourse.bass_utils import run_bass_kernel_spmd

T = 8192
D = 32
TQ = 4096
NT = TQ // 128
NCH = T // 128
NG = NT // 4
F32 = mybir.dt.float32
F32R = mybir.dt.float32r
AF = mybir.ActivationFunctionType
OP = mybir.AluOpType

# packed input column maps
W32_COLS = 384 + 384 + 3 + 3 + 32 + NCH   # wq | wk | bq | bk | id4 | t2p
WR_COLS = 4 * NCH                         # v4n: per chunk [vr0 vr1 ve0 ve1]


def _s4(xT):
    """(32, N) -> (128, N//4): 128-col chunk c -> partitions 32*(c%4),
    cols 128*(c//4)."""
    d, N = xT.shape
    nch = N // 128
    out = np.zeros((128, N // 4), dtype=xT.dtype)
    for c in range(nch):
        out[32 * (c % 4):32 * (c % 4) + 32,
            128 * (c // 4):128 * (c // 4) + 128] = xT[:, 128 * c:128 * c + 128]
    return out


def _band_meta(t1_all, t2_all):
    w_raw = np.full(NT, T, dtype=np.int64)
    for t1 in t1_all:
        for t2 in t2_all:
            r_min = np.searchsorted(t2, t1[::128], side="right")
            w_raw = np.minimum(w_raw, (r_min // 128) * 128)
    e = np.zeros(NT, dtype=np.int64)
    for t1 in t1_all:
        for t2 in t2_all:
            r_max = np.searchsorted(t2, t1[127::128], side="right")
            e = np.maximum(e, (r_max + 127) // 128)
    wc = w_raw // 128
    e = np.maximum(e, wc + 1)
    e = np.minimum(np.maximum.accumulate(e), NCH)
    wc = np.minimum(wc, e - 1)
    assert np.all(np.diff(wc) >= 0) and np.all(np.diff(e) >= 0)
    for t1 in t1_all:
        for t2 in t2_all:
            r_min = np.searchsorted(t2, t1[::128], side="right")
            r_max = np.searchsorted(t2, t1[127::128], side="right")
            assert np.all(wc * 128 <= r_min) and np.all(r_max <= e * 128)
    return [int(x) for x in wc], [int(x) for x in e]


def _pieces_meta(wc, e, t1_all, t2_all):
    pieces = []
    for c in range(NCH):
        tiles = [I for I in range(NT) if wc[I] <= c < e[I]]
        if not tiles:
            continue
        lo, ihi = tiles[0], tiles[-1] + 1
        qlo = 128 * lo
        qmin = TQ
        for t1 in t1_all:
            for t2 in t2_all:
                qmin = min(qmin, int(np.searchsorted(t1, t2[128 * c])))
        qlo = max(qlo, (qmin // 64) * 64)
        qlo = min(qlo, 128 * ihi - 64)
        pieces.append((c, qlo, lo, ihi))
    return pieces


def _build(wc, e, pieces):
    nc = Bacc("TRN2")

    xk = nc.dram_tensor("xk", [128, T // 4], F32, kind="ExternalInput")
    xq = nc.dram_tensor("xq", [128, TQ // 4], F32, kind="ExternalInput")
    wp32 = nc.dram_tensor("wp32", [128, W32_COLS], F32, kind="ExternalInput")
    wpr = nc.dram_tensor("wpr", [128, WR_COLS], F32R, kind="ExternalInput")
    t1 = nc.dram_tensor("t1", [1, TQ], F32, kind="ExternalInput")
    out = nc.dram_tensor("out", [2, TQ], F32, kind="ExternalOutput")
    hdbg = nc.dram_tensor("hdbg", [32, 4 * NT], F32, kind="ExternalOutput")

    maxw = max(wc)
    segs = []
    prev = 0
    for I in range(NT):
        if wc[I] > prev:
            segs.append((prev, wc[I]))
            prev = wc[I]
    nseg = len(segs)
    segidx = []  # tile I -> index into hsball col groups (0 = zero-H)
    prev = 0
    si = 0
    for I in range(NT):
        if wc[I] > prev:
            si += 1
            prev = wc[I]
        segidx.append(si)

    gparts = {g: [] for g in range(NG)}
    for idx, (c, qlo, lo, ihi) in enumerate(pieces):
        for g in range((qlo // 512), (ihi * 128 - 1) // 512 + 1):
            a = max(qlo, 512 * g)
            b = min(128 * ihi, 512 * g + 512)
            gparts[g].append((idx, a, b))

    with TileContext(nc) as tc:
        with tc.tile_pool(name="cst", bufs=1) as cst, \
             tc.tile_pool(name="big", bufs=1) as big:

            wp32_s = cst.tile([128, W32_COLS], F32)
            wpr_s = cst.tile([128, WR_COLS], F32R)
            wq_s = wp32_s[:, 0:384]
            wk_s = wp32_s[:, 384:768]
            bq_s = wp32_s[:, 768:771]
            bk_s = wp32_s[:, 771:774]
            id4_s = wp32_s[:, 774:806]
            t2p_s = wp32_s[:, 806:806 + NCH]
            v4n_s = wpr_s[:, 0:4 * NCH]

            t1b_s = big.tile([128, TQ], F32, tag="t1b")
            xk_a = big.tile([128, T // 8], F32, tag="xka")
            xk_b = big.tile([128, T // 8], F32, tag="xkb")
            xq_s = big.tile([128, TQ // 4], F32, tag="xq")
            kt_s = big.tile([128, T // 4], F32, tag="kt")
            qts32 = big.tile([128, TQ // 4], F32, tag="qts32")
            qrep32 = big.tile([128, TQ], F32, tag="qrep32")
            qrepr = big.tile([128, TQ], F32R, tag="qrepr")
            qrepe = big.tile([128, TQ], F32R, tag="qrepe")
            ktr = big.tile([128, T // 4], F32R, tag="ktr")
            knr = big.tile([128, 32 * NCH], F32R, tag="knr")
            kne = big.tile([128, 32 * NCH], F32R, tag="kne")
            hsball = cst.tile([32, 4 * (NT + 1)], F32)
            hsbrep = cst.tile([128, 4 * (NT + 1)], F32)
            zsb = cst.tile([2, TQ], F32)

            nc.sync.dma_start(wp32_s[:], wp32[:])
            nc.sync.dma_start(xq_s[:], xq[:])
            half = T // 8
            nc.scalar.dma_start(xk_a[:], xk[:, 0:half])
            nc.scalar.dma_start(xk_b[:], xk[:, half:])
            nc.sync.dma_start(wpr_s[:], wpr[:])
            nc.gpsimd.dma_start(t1b_s[0:1, :], t1[:])
            p = 1
            while p < 128:
                nc.gpsimd.dma_start(t1b_s[p:2 * p, :], t1b_s[0:p, :])
                p *= 2

            # ---------------- MLPs (block-diag 128-contract) -------------
            with tc.tile_pool(name="mlp", bufs=3, space="PSUM") as mlp, \
                 tc.tile_pool(name="hbuf", bufs=2) as hbuf:

                def run_mlp(x_parts, w_s, b_s, ngrp, dst, dt):
                    h_prev = None
                    for l in range(3):
                        h_next = dst if l == 2 else hbuf.tile(
                            [128, ngrp * 512], dt, tag=f"h{ngrp}",
                            name=f"h{ngrp}_{l}")
                        for G in range(ngrp):
                            if l == 0:
                                npart = len(x_parts)
                                gper = ngrp // npart
                                src_ap = x_parts[G // gper][
                                    :, 512 * (G % gper):512 * (G % gper) + 512]
                            else:
                                src_ap = h_prev[:, 512 * G:512 * G + 512]
                            pt = mlp.tile([128, 512], F32, tag="mlp")
                            nc.tensor.matmul(
                                pt[:], w_s[:, 128 * l:128 * l + 128],
                                src_ap,
                                start=True, stop=True)
                            o = h_next[:, 512 * G:512 * G + 512]
                            if G % 2 == 0:
                                if l < 2:
                                    nc.scalar.activation(
                                        o, pt[:], AF.Relu, bias=b_s[:, l:l + 1])
                                else:
                                    nc.scalar.activation(
                                        o, pt[:], AF.Identity,
                                        bias=b_s[:, l:l + 1])
                            else:
                                if l < 2:
                                    nc.vector.tensor_scalar(
                                        o, pt[:], b_s[:, l:l + 1], 0.0,
                                        OP.add, OP.max)
                                else:
                                    nc.vector.tensor_scalar(
                                        o, pt[:], b_s[:, l:l + 1], None,
                                        OP.add)
                        h_prev = h_next

                run_mlp([xq_s], wq_s, bq_s, 2, qts32, F32)
                run_mlp([xk_a, xk_b], wk_s, bk_s, 4, kt_s, F32)

            # replicated flat Q^T (fp32), issued off-ACT
            for b in range(4):
                for k in range(4):
                    nc.gpsimd.dma_start(
                        qrep32[32 * b:32 * b + 32, :].rearrange(
                            "d (t c) -> d t c", c=128)[:, k::4, :],
                        qts32[32 * k:32 * k + 32, :].rearrange(
                            "d (t c) -> d t c", c=128))

            # round/residual copies for the fp32r band path
            for j in range(4):
                sl = slice(512 * j, 512 * j + 512)
                nc.scalar.activation(ktr[:, sl], kt_s[:, sl], AF.Copy)
            for j in range(8):
                sl = slice(512 * j, 512 * j + 512)
                if j % 2 == 0:
                    nc.scalar.activation(qrepr[:, sl], qrep32[:, sl], AF.Copy)
                else:
                    nc.vector.tensor_copy(qrepr[:, sl], qrep32[:, sl])
                nc.gpsimd.tensor_tensor(
                    qrepe[:, sl], qrep32[:, sl],
                    qrepr[:, sl].bitcast(F32), OP.subtract)

            # knat transposes (fp32) + exact split into knr/kne (fp32r)
            with tc.tile_pool(name="knT", bufs=1, space="PSUM") as knT:
                knrv = knr[:, :].rearrange("p (c d) -> p c d", d=32)
                knev = kne[:, :].rearrange("p (c d) -> p c d", d=32)
                for half in range(2):
                    pts = [knT.tile([128, 256], F32, tag=f"knT{q}",
                                    name=f"knT{q}") for q in range(4)]
                    for i in range(8):
                        for q in range(4):
                            col = 8 * half + i
                            nc.tensor.matmul(
                                pts[q][:, 32 * i:32 * i + 32],
                                kt_s[32 * q:32 * q + 32,
                                     128 * col:128 * col + 128],
                                id4_s[32 * q:32 * q + 32, :],
                                start=True, stop=True,
                                tile_position=(32 * q, 0))
                    for q in range(4):
                        pv = pts[q][:].rearrange("p (c d) -> p c d", d=32)
                        orr = knrv[:, q + 4 * 8 * half::4, :][:, 0:8, :]
                        oe = knev[:, q + 4 * 8 * half::4, :][:, 0:8, :]
                        nc.scalar.activation(orr, pv, AF.Copy)
                        nc.vector.scalar_tensor_tensor(
                            oe, orr.bitcast(F32), -1.0, pv,
                            OP.mult, OP.add)

            # prefix chain (fp32r): per-segment sums into one wide PSUM
            # tile, then 4 strided DVE scans produce all H prefixes
            with tc.tile_pool(name="hps", bufs=1, space="PSUM") as hps:
                dpsw = hps.tile([32, 4 * nseg], F32, tag="dh")
                for s, (p0, p1) in enumerate(segs):
                    for c in range(p0, p1):
                        nc.tensor.matmul(
                            dpsw[:, 4 * s:4 * s + 4],
                            knr[:, 32 * c:32 * c + 32],
                            v4n_s[:, 4 * c:4 * c + 4],
                            start=(c == p0), stop=False,
                            skip_group_check=True)
                        nc.tensor.matmul(
                            dpsw[:, 4 * s:4 * s + 2],
                            kne[:, 32 * c:32 * c + 32],
                            v4n_s[:, 4 * c:4 * c + 2],
                            start=False, stop=(c == p1 - 1),
                            skip_group_check=True)
                nc.vector.memset(hsball[:, 0:4], 0)
                for v in range(4):
                    nc.vector.tensor_tensor_scan(
                        hsball[:, 4 + v:4 + 4 * nseg:4], dpsw[:, v::4],
                        wp32_s[0:32, 0:nseg], 0.0, OP.add, OP.bypass)

            # replicate H table (fp32) to all 4 partition blocks
            for q in range(4):
                nc.gpsimd.dma_start(hsbrep[32 * q:32 * q + 32, :], hsball[:])


            # ---------------- band ----------------
            with tc.tile_pool(name="stp", bufs=4, space="PSUM") as stp, \
                 tc.tile_pool(name="zob", bufs=1, space="PSUM") as zob, \
                 tc.tile_pool(name="zop", bufs=1, space="PSUM") as zop, \
                 tc.tile_pool(name="smp", bufs=12) as smp, \
             tc.tile_pool(name="zps", bufs=2) as zpsp:

                # rotated fp32 prefix-zo straight into zoB rows 0:4
                zoBs = {}

                def emit_prefix_batch(gs):
                    for g in gs:
                        zoBs[g] = zob.tile([4, 512], F32, tag=f"zoB{g % 4}",
                                           name=f"zoB{g % 4}")
                    for t in range(4):
                        for g in gs:
                            I = 4 * g + t
                            si = segidx[I]
                            qq = g % 4
                            nc.tensor.matmul(
                                zoBs[g][:, 128 * t:128 * t + 128],
                                hsbrep[32 * qq:32 * qq + 32,
                                       4 * si:4 * si + 4],
                                qrep32[32 * qq:32 * qq + 32,
                                       128 * I:128 * I + 128],
                                start=(t == 0), stop=False,
                                tile_position=(32 * qq, 0),
                                skip_group_check=True)

                emit_prefix_batch(range(0, 4))
                made = {}
                for g in range(NG):
                    if g == 4:
                        emit_prefix_batch(range(4, 8))
                    zoB = zoBs[g]
                    for (idx, a, b) in gparts[g]:
                        if idx in made:
                            continue
                        c, qlo, lo, ihi = pieces[idx]
                        wd = 128 * ihi - qlo
                        q = c % 4
                        stb = stp.tile([128, 512], F32, tag="st")
                        kslice = ktr[32 * q:32 * q + 32,
                                     128 * (c // 4):128 * (c // 4) + 128]
                        nc.tensor.matmul(
                            stb[:, 0:wd], kslice,
                            qrepr[32 * q:32 * q + 32, qlo:128 * ihi],
                            start=True, stop=False,
                            tile_position=(32 * q, 0))
                        nc.tensor.matmul(
                            stb[:, 0:wd], kslice,
                            qrepe[32 * q:32 * q + 32, qlo:128 * ihi],
                            start=False, stop=True,
                            tile_position=(32 * q, 0),
                            skip_group_check=True)
                        smt = smp.tile([128, 512], F32R, tag="smt")
                        nc.vector.scalar_tensor_tensor(
                            smt[:, 0:wd], t1b_s[:, qlo:128 * ihi],
                            t2p_s[:, c:c + 1], stb[:, 0:wd],
                            OP.is_ge, OP.mult)
                        made[idx] = smt
                    nparts = len(gparts[g])
                    assert nparts > 0
                    for i, (idx, a, b) in enumerate(gparts[g]):
                        c, qlo, lo, ihi = pieces[idx]
                        smt = made[idx]
                        nc.tensor.matmul(
                            zoB[:, a - 512 * g:b - 512 * g],
                            v4n_s[:, 4 * c:4 * c + 4],
                            smt[:, a - qlo:b - qlo],
                            start=False, stop=(i == nparts - 1),
                            skip_group_check=True)
                    zb4 = zpsp.tile([4, 512], F32, tag="zb4")
                    nc.scalar.activation(zb4[:], zoB[:], AF.Copy)
                    zbs = zpsp.tile([2, 512], F32, tag="zbs")
                    nc.gpsimd.dma_start(zbs[:], zb4[2:4, :])
                    eng = nc.vector if g >= 6 else nc.gpsimd
                    eng.tensor_tensor(
                        zsb[:, 512 * g:512 * g + 512], zb4[0:2, :], zbs[:],
                        OP.add)
                    nc.sync.dma_start(out[:, 512 * g:512 * g + 512],
                                      zsb[:, 512 * g:512 * g + 512])

            nc.sync.dma_start(hdbg[:], hsball[:, 0:4 * NT])
    nc.finalize()
    return nc


_CACHE = {}
LAST_RESULTS = None


def kernel(m1, m2, m3, m4, Wq, bq, Wk, bk):
    mods = [np.asarray(m)[0, 0].astype(np.float32) for m in (m1, m2, m3, m4)]
    Wq, bq, Wk, bk = (np.asarray(a, dtype=np.float32) for a in (Wq, bq, Wk, bk))
    t2s = [m[:, -1].copy() for m in mods]
    t1g = mods[0][:, -1].copy()

    def qsel(h):
        idx = np.arange(TQ)
        gt = 2 * (idx // 128) + h
        return gt * 128 + (idx % 128)

    sels = [qsel(0), qsel(1)]
    t1_locals = [t1g[s] for s in sels]
    wc, e = _band_meta(t1_locals, t2s)
    pieces = _pieces_meta(wc, e, t1_locals, t2s)

    key = (tuple(wc), tuple(e), tuple(p[1] for p in pieces))
    if key not in _CACHE:
        _CACHE[key] = _build(wc, e, pieces)
    nc = _CACHE[key]

    def chop11(x):
        # zero low 12 mantissa bits: exactly representable in fp32r (11b)
        xm = np.ascontiguousarray(x, np.float32).view(np.uint32)
        return (xm & np.uint32(0xFFFFF000)).view(np.float32).copy()

    def blockdiag(W):
        out = np.zeros((128, 384), dtype=np.float32)
        for l in range(3):
            for k in range(4):
                out[32 * k:32 * k + 32,
                    128 * l + 32 * k:128 * l + 32 * k + 32] = W[l]
        return out

    bq_in = np.tile(bq.T, (4, 1)).astype(np.float32)
    bk_in = np.tile(bk.T, (4, 1)).astype(np.float32)
    id4_in = np.tile(np.eye(32, dtype=np.float32), (4, 1))

    in_maps = []
    for core in range(8):
        mod, h = core // 2, core % 2
        x = mods[mod]
        t2 = t2s[mod]
        xk_in = _s4(np.ascontiguousarray(x.T))
        xq_l = mods[0][sels[h]]
        xq_in = _s4(np.ascontiguousarray(xq_l.T))
        t1_in = t1_locals[h].reshape(1, TQ).astype(np.float32)
        t2p_in = np.ascontiguousarray(t2.reshape(NCH, 128).T)
        v2n_in = np.ascontiguousarray(
            x[:, :2].reshape(NCH, 128, 2).transpose(1, 0, 2)
            .reshape(128, 2 * NCH))
        v2nr_in = chop11(v2n_in)
        v2ne_in = v2n_in - v2nr_in
        v4n_in = np.zeros((128, 4 * NCH), dtype=np.float32)
        v4n_in[:, 0::4] = v2nr_in[:, 0::2]
        v4n_in[:, 1::4] = v2nr_in[:, 1::2]
        v4n_in[:, 2::4] = v2ne_in[:, 0::2]
        v4n_in[:, 3::4] = v2ne_in[:, 1::2]
        wp32_in = np.concatenate(
            [blockdiag(Wq), blockdiag(Wk), bq_in, bk_in, id4_in, t2p_in],
            axis=1)
        wpr_in = v4n_in
        in_maps.append({
            "xk": xk_in, "xq": xq_in, "wp32": wp32_in, "wpr": wpr_in,
            "t1": t1_in,
        })

    import os as _os
    trace = bool(_os.environ.get("KERNEL_TRACE"))
    res = run_bass_kernel_spmd(nc, in_maps, core_ids=list(range(8)),
                               trace=trace)
    global LAST_RESULTS
    LAST_RESULTS = res

    y = np.zeros((T, 2), dtype=np.float32)
    for core in range(8):
        mod, h = core // 2, core % 2
        zt = res.results[core]["out"]
        y[sels[h]] += zt.T
    return y[None, :, :]


# revision 26
# speedup vs baseline: 1.1717x; 1.0020x over previous
"""Trainium2 Bass kernel for sparse_attention problem nn_CAMD_73229192397362.

v4 precision model (HW-validated: fp32r = round-to-nearest ~11-bit, but
the tolerance needs ~15+ bits on every path feeding the 7e4-magnitude
accumulations):
  - Both MLPs, band S^T, knat transposes, H snapshots and prefix-zo run
    in fp32.
  - The prefix chain K^T V runs as THREE fp32r matmuls per chunk
    (Kr Vr + Kr Ve + Ke Vr) with exact splits: V split on host,
    K split on-chip from the fp32 knat (round-copy + subtract).
  - The band zo runs fp32r on the fp32-exact masked S (smt) and Vr; its
    residuals are per-key random and average out over the band.

Structure per core (8 = 4 modalities x 2 query half-sets):
  stacked s4 layout (chunk c -> partitions 32*(c%4), cols 128*(c//4));
  block-diag 128-contract MLPs; rotated (tile_position) 32-contract
  band S^T and knat transposes; per-tile prefix H folded in via rotated
  fp32 prefix-zo into 4 PSUM banks (zoP), combined with the band zo
  accumulator (zoB) on DVE at the end of each 512-query group.
"""

import numpy as np

import concourse.bass as bass
from concourse.bacc import Bacc
import concourse.mybir as mybir
from concourse.tile import TileContext
from concourse.bass_utils import run_bass_kernel_spmd

T = 8192
D = 32
TQ = 4096
NT = TQ // 128
NCH = T // 128
NG = NT // 4
F32 = mybir.dt.float32
F32R = mybir.dt.float32r
AF = mybir.ActivationFunctionType
OP = mybir.AluOpType

# packed input column maps
W32_COLS = 384 + 384 + 3 + 3 + 32 + NCH   # wq | wk | bq | bk | id4 | t2p
WR_COLS = 4 * NCH                         # v4n: per chunk [vr0 vr1 ve0 ve1]


def _s4(xT):
    """(32, N) -> (128, N//4): 128-col chunk c -> partitions 32*(c%4),
    cols 128*(c//4)."""
    d, N = xT.shape
    nch = N // 128
    out = np.zeros((128, N // 4), dtype=xT.dtype)
    for c in range(nch):
        out[32 * (c % 4):32 * (c % 4) + 32,
            128 * (c // 4):128 * (c // 4) + 128] = xT[:, 128 * c:128 * c + 128]
    return out


def _band_meta(t1_all, t2_all):
    w_raw = np.full(NT, T, dtype=np.int64)
    for t1 in t1_all:
        for t2 in t2_all:
            r_min = np.searchsorted(t2, t1[::128], side="right")
            w_raw = np.minimum(w_raw, (r_min // 128) * 128)
    e = np.zeros(NT, dtype=np.int64)
    for t1 in t1_all:
        for t2 in t2_all:
            r_max = np.searchsorted(t2, t1[127::128], side="right")
            e = np.maximum(e, (r_max + 127) // 128)
    wc = w_raw // 128
    e = np.maximum(e, wc + 1)
    e = np.minimum(np.maximum.accumulate(e), NCH)
    wc = np.minimum(wc, e - 1)
    assert np.all(np.diff(wc) >= 0) and np.all(np.diff(e) >= 0)
    for t1 in t1_all:
        for t2 in t2_all:
            r_min = np.searchsorted(t2, t1[::128], side="right")
            r_max = np.searchsorted(t2, t1[127::128], side="right")
            assert np.all(wc * 128 <= r_min) and np.all(r_max <= e * 128)
    return [int(x) for x in wc], [int(x) for x in e]


def _pieces_meta(wc, e, t1_all, t2_all):
    pieces = []
    for c in range(NCH):
        tiles = [I for I in range(NT) if wc[I] <= c < e[I]]
        if not tiles:
            continue
        lo, ihi = tiles[0], tiles[-1] + 1
        qlo = 128 * lo
        qmin = TQ
        for t1 in t1_all:
            for t2 in t2_all:
                qmin = min(qmin, int(np.searchsorted(t1, t2[128 * c])))
        qlo = max(qlo, (qmin // 64) * 64)
        qlo = min(qlo, 128 * ihi - 64)
        pieces.append((c, qlo, lo, ihi))
    return pieces


def _build(wc, e, pieces):
    nc = Bacc("TRN2")

    xk = nc.dram_tensor("xk", [128, T // 4], F32, kind="ExternalInput")
    xq = nc.dram_tensor("xq", [128, TQ // 4], F32, kind="ExternalInput")
    wp32 = nc.dram_tensor("wp32", [128, W32_COLS], F32, kind="ExternalInput")
    wpr = nc.dram_tensor("wpr", [128, WR_COLS], F32R, kind="ExternalInput")
    t1 = nc.dram_tensor("t1", [1, TQ], F32, kind="ExternalInput")
    out = nc.dram_tensor("out", [2, TQ], F32, kind="ExternalOutput")
    hdbg = nc.dram_tensor("hdbg", [32, 4 * NT], F32, kind="ExternalOutput")

    maxw = max(wc)
    segs = []
    prev = 0
    for I in range(NT):
        if wc[I] > prev:
            segs.append((prev, wc[I]))
            prev = wc[I]
    nseg = len(segs)
    segidx = []  # tile I -> index into hsball col groups (0 = zero-H)
    prev = 0
    si = 0
    for I in range(NT):
        if wc[I] > prev:
            si += 1
            prev = wc[I]
        segidx.append(si)

    gparts = {g: [] for g in range(NG)}
    for idx, (c, qlo, lo, ihi) in enumerate(pieces):
        for g in range((qlo // 512), (ihi * 128 - 1) // 512 + 1):
            a = max(qlo, 512 * g)
            b = min(128 * ihi, 512 * g + 512)
            gparts[g].append((idx, a, b))

    with TileContext(nc) as tc:
        with tc.tile_pool(name="cst", bufs=1) as cst, \
             tc.tile_pool(name="big", bufs=1) as big:

            wp32_s = cst.tile([128, W32_COLS], F32)
            wpr_s = cst.tile([128, WR_COLS], F32R)
            wq_s = wp32_s[:, 0:384]
            wk_s = wp32_s[:, 384:768]
            bq_s = wp32_s[:, 768:771]
            bk_s = wp32_s[:, 771:774]
            id4_s = wp32_s[:, 774:806]
            t2p_s = wp32_s[:, 806:806 + NCH]
            v4n_s = wpr_s[:, 0:4 * NCH]

            t1b_s = big.tile([128, TQ], F32, tag="t1b")
            xk_a = big.tile([128, T // 8], F32, tag="xka")
            xk_b = big.tile([128, T // 8], F32, tag="xkb")
            xq_a = big.tile([128, TQ // 8], F32, tag="xqa")
            xq_b = big.tile([128, TQ // 8], F32, tag="xqb")
            kt_s = big.tile([128, T // 4], F32, tag="kt")
            qts32 = big.tile([128, TQ // 4], F32, tag="qts32")
            qrep32 = big.tile([128, TQ], F32, tag="qrep32")
            qrepr = big.tile([128, TQ], F32R, tag="qrepr")
            qrepe = big.tile([128, TQ], F32R, tag="qrepe")
            ktr = big.tile([128, T // 4], F32R, tag="ktr")
            knr = big.tile([128, 32 * NCH], F32R, tag="knr")
            kne = big.tile([128, 32 * NCH], F32R, tag="kne")
            hsball = cst.tile([32, 4 * (NT + 1)], F32)
            hsbrep = cst.tile([128, 4 * (NT + 1)], F32)
            zsb = cst.tile([2, TQ], F32)

            nc.sync.dma_start(wp32_s[:], wp32[:])
            nc.sync.dma_start(xq_a[:], xq[:, 0:TQ // 8])
            nc.sync.dma_start(xq_b[:], xq[:, TQ // 8:])
            half = T // 8
            nc.scalar.dma_start(xk_a[:], xk[:, 0:half])
            nc.scalar.dma_start(xk_b[:], xk[:, half:])
            nc.sync.dma_start(wpr_s[:], wpr[:])
            nc.gpsimd.dma_start(t1b_s[0:1, :], t1[:])
            p = 1
            while p < 128:
                nc.gpsimd.dma_start(t1b_s[p:2 * p, :], t1b_s[0:p, :])
                p *= 2

            # ---------------- MLPs (block-diag 128-contract) -------------
            with tc.tile_pool(name="mlp", bufs=3, space="PSUM") as mlp, \
                 tc.tile_pool(name="hbuf", bufs=2) as hbuf:

                def run_mlp(x_parts, w_s, b_s, ngrp, dst, dt):
                    h_prev = None
                    for l in range(3):
                        h_next = dst if l == 2 else hbuf.tile(
                            [128, ngrp * 512], dt, tag=f"h{ngrp}",
                            name=f"h{ngrp}_{l}")
                        for G in range(ngrp):
                            if l == 0:
                                npart = len(x_parts)
                                gper = ngrp // npart
                                src_ap = x_parts[G // gper][
                                    :, 512 * (G % gper):512 * (G % gper) + 512]
                            else:
                                src_ap = h_prev[:, 512 * G:512 * G + 512]
                            pt = mlp.tile([128, 512], F32, tag="mlp")
                            nc.tensor.matmul(
                                pt[:], w_s[:, 128 * l:128 * l + 128],
                                src_ap,
                                start=True, stop=True)
                            o = h_next[:, 512 * G:512 * G + 512]
                            if G % 2 == 0:
                                if l < 2:
                                    nc.scalar.activation(
                                        o, pt[:], AF.Relu, bias=b_s[:, l:l + 1])
                                else:
                                    nc.scalar.activation(
                                        o, pt[:], AF.Identity,
                                        bias=b_s[:, l:l + 1])
                            else:
                                if l < 2:
                                    nc.vector.tensor_scalar(
                                        o, pt[:], b_s[:, l:l + 1], 0.0,
                                        OP.add, OP.max)
                                else:
                                    nc.vector.tensor_scalar(
                                        o, pt[:], b_s[:, l:l + 1], None,
                                        OP.add)
                        h_prev = h_next

                run_mlp([xq_a, xq_b], wq_s, bq_s, 2, qts32, F32)
                run_mlp([xk_a, xk_b], wk_s, bk_s, 4, kt_s, F32)

            # replicated flat Q^T (fp32), issued off-ACT
            for b in range(4):
                for k in range(4):
                    nc.gpsimd.dma_start(
                        qrep32[32 * b:32 * b + 32, :].rearrange(
                            "d (t c) -> d t c", c=128)[:, k::4, :],
                        qts32[32 * k:32 * k + 32, :].rearrange(
                            "d (t c) -> d t c", c=128))

            # round/residual copies for the fp32r band path
            for j in range(4):
                sl = slice(512 * j, 512 * j + 512)
                nc.scalar.activation(ktr[:, sl], kt_s[:, sl], AF.Copy)
            for j in range(8):
                sl = slice(512 * j, 512 * j + 512)
                if j % 2 == 0:
                    nc.scalar.activation(qrepr[:, sl], qrep32[:, sl], AF.Copy)
                else:
                    nc.vector.tensor_copy(qrepr[:, sl], qrep32[:, sl])
                nc.gpsimd.tensor_tensor(
                    qrepe[:, sl], qrep32[:, sl],
                    qrepr[:, sl].bitcast(F32), OP.subtract)

            # knat transposes (fp32) + exact split into knr/kne (fp32r)
            with tc.tile_pool(name="knT", bufs=1, space="PSUM") as knT:
                knrv = knr[:, :].rearrange("p (c d) -> p c d", d=32)
                knev = kne[:, :].rearrange("p (c d) -> p c d", d=32)
                for half in range(2):
                    pts = [knT.tile([128, 256], F32, tag=f"knT{q}",
                                    name=f"knT{q}") for q in range(4)]
                    for i in range(8):
                        for q in range(4):
                            col = 8 * half + i
                            nc.tensor.matmul(
                                pts[q][:, 32 * i:32 * i + 32],
                                kt_s[32 * q:32 * q + 32,
                                     128 * col:128 * col + 128],
                                id4_s[32 * q:32 * q + 32, :],
                                start=True, stop=True,
                                tile_position=(32 * q, 0))
                    for q in range(4):
                        pv = pts[q][:].rearrange("p (c d) -> p c d", d=32)
                        orr = knrv[:, q + 4 * 8 * half::4, :][:, 0:8, :]
                        oe = knev[:, q + 4 * 8 * half::4, :][:, 0:8, :]
                        nc.scalar.activation(orr, pv, AF.Copy)
                        nc.vector.scalar_tensor_tensor(
                            oe, orr.bitcast(F32), -1.0, pv,
                            OP.mult, OP.add)

            # prefix chain (fp32r): per-segment sums into one wide PSUM
            # tile, then 4 strided DVE scans produce all H prefixes
            with tc.tile_pool(name="hps", bufs=1, space="PSUM") as hps:
                dpsw = hps.tile([32, 4 * nseg], F32, tag="dh")
                for s, (p0, p1) in enumerate(segs):
                    for c in range(p0, p1):
                        nc.tensor.matmul(
                            dpsw[:, 4 * s:4 * s + 4],
                            knr[:, 32 * c:32 * c + 32],
                            v4n_s[:, 4 * c:4 * c + 4],
                            start=(c == p0), stop=False,
                            skip_group_check=True)
                        nc.tensor.matmul(
                            dpsw[:, 4 * s:4 * s + 2],
                            kne[:, 32 * c:32 * c + 32],
                            v4n_s[:, 4 * c:4 * c + 2],
                            start=False, stop=(c == p1 - 1),
                            skip_group_check=True)
                nc.vector.memset(hsball[:, 0:4], 0)
                for v in range(4):
                    nc.vector.tensor_tensor_scan(
                        hsball[:, 4 + v:4 + 4 * nseg:4], dpsw[:, v::4],
                        wp32_s[0:32, 0:nseg], 0.0, OP.add, OP.bypass)

            # replicate H table (fp32) to all 4 partition blocks
            for q in range(4):
                nc.gpsimd.dma_start(hsbrep[32 * q:32 * q + 32, :], hsball[:])


            # ---------------- band ----------------
            with tc.tile_pool(name="stp", bufs=4, space="PSUM") as stp, \
                 tc.tile_pool(name="zob", bufs=1, space="PSUM") as zob, \
                 tc.tile_pool(name="zop", bufs=1, space="PSUM") as zop, \
                 tc.tile_pool(name="smp", bufs=12) as smp, \
             tc.tile_pool(name="zps", bufs=2) as zpsp:

                # rotated fp32 prefix-zo straight into zoB rows 0:4
                zoBs = {}

                def emit_prefix_batch(gs):
                    for g in gs:
                        zoBs[g] = zob.tile([4, 512], F32, tag=f"zoB{g % 4}",
                                           name=f"zoB{g % 4}")
                    for t in range(4):
                        for g in gs:
                            I = 4 * g + t
                            si = segidx[I]
                            qq = g % 4
                            nc.tensor.matmul(
                                zoBs[g][:, 128 * t:128 * t + 128],
                                hsbrep[32 * qq:32 * qq + 32,
                                       4 * si:4 * si + 4],
                                qrep32[32 * qq:32 * qq + 32,
                                       128 * I:128 * I + 128],
                                start=(t == 0), stop=False,
                                tile_position=(32 * qq, 0),
                                skip_group_check=True)

                made = {}
                for g in range(NG):
                    zoB = None
                    for (idx, a, b) in gparts[g]:
                        if idx in made:
                            continue
                        c, qlo, lo, ihi = pieces[idx]
                        wd = 128 * ihi - qlo
                        q = c % 4
                        stb = stp.tile([128, 512], F32, tag="st")
                        kslice = ktr[32 * q:32 * q + 32,
                                     128 * (c // 4):128 * (c // 4) + 128]
                        nc.tensor.matmul(
                            stb[:, 0:wd], kslice,
                            qrepr[32 * q:32 * q + 32, qlo:128 * ihi],
                            start=True, stop=False,
                            tile_position=(32 * q, 0))
                        nc.tensor.matmul(
                            stb[:, 0:wd], kslice,
                            qrepe[32 * q:32 * q + 32, qlo:128 * ihi],
                            start=False, stop=True,
                            tile_position=(32 * q, 0),
                            skip_group_check=True)
                        smt = smp.tile([128, 512], F32R, tag="smt")
                        nc.vector.scalar_tensor_tensor(
                            smt[:, 0:wd], t1b_s[:, qlo:128 * ihi],
                            t2p_s[:, c:c + 1], stb[:, 0:wd],
                            OP.is_ge, OP.mult)
                        made[idx] = smt
                    if g == 0:
                        emit_prefix_batch(range(0, 4))
                    elif g == 4:
                        emit_prefix_batch(range(4, 8))
                    zoB = zoBs[g]
                    nparts = len(gparts[g])
                    assert nparts > 0
                    for i, (idx, a, b) in enumerate(gparts[g]):
                        c, qlo, lo, ihi = pieces[idx]
                        smt = made[idx]
                        nc.tensor.matmul(
                            zoB[:, a - 512 * g:b - 512 * g],
                            v4n_s[:, 4 * c:4 * c + 4],
                            smt[:, a - qlo:b - qlo],
                            start=False, stop=(i == nparts - 1),
                            skip_group_check=True)
                    zb4 = zpsp.tile([4, 512], F32, tag="zb4")
                    nc.scalar.activation(zb4[:], zoB[:], AF.Copy)
                    zbs = zpsp.tile([2, 512], F32, tag="zbs")
                    nc.gpsimd.dma_start(zbs[:], zb4[2:4, :])
                    eng = nc.vector if g >= 6 else nc.gpsimd
                    eng.tensor_tensor(
                        zsb[:, 512 * g:512 * g + 512], zb4[0:2, :], zbs[:],
                        OP.add)
                    nc.sync.dma_start(out[:, 512 * g:512 * g + 512],
                                      zsb[:, 512 * g:512 * g + 512])

            nc.sync.dma_start(hdbg[:], hsball[:, 0:4 * NT])
    nc.finalize()
    return nc


_CACHE = {}
LAST_RESULTS = None


def kernel(m1, m2, m3, m4, Wq, bq, Wk, bk):
    mods = [np.asarray(m)[0, 0].astype(np.float32) for m in (m1, m2, m3, m4)]
    Wq, bq, Wk, bk = (np.asarray(a, dtype=np.float32) for a in (Wq, bq, Wk, bk))
    t2s = [m[:, -1].copy() for m in mods]
    t1g = mods[0][:, -1].copy()

    def qsel(h):
        idx = np.arange(TQ)
        gt = 2 * (idx // 128) + h
        return gt * 128 + (idx % 128)

    sels = [qsel(0), qsel(1)]
    t1_locals = [t1g[s] for s in sels]
    wc, e = _band_meta(t1_locals, t2s)
    pieces = _pieces_meta(wc, e, t1_locals, t2s)

    key = (tuple(wc), tuple(e), tuple(p[1] for p in pieces))
    if key not in _CACHE:
        _CACHE[key] = _build(wc, e, pieces)
    nc = _CACHE[key]

    def chop11(x):
        # zero low 12 mantissa bits: exactly representable in fp32r (11b)
        xm = np.ascontiguousarray(x, np.float32).view(np.uint32)
        return (xm & np.uint32(0xFFFFF000)).view(np.float32).copy()

    def blockdiag(W):
        out = np.zeros((128, 384), dtype=np.float32)
        for l in range(3):
            for k in range(4):
                out[32 * k:32 * k + 32,
                    128 * l + 32 * k:128 * l + 32 * k + 32] = W[l]
        return out

    bq_in = np.tile(bq.T, (4, 1)).astype(np.float32)
    bk_in = np.tile(bk.T, (4, 1)).astype(np.float32)
    id4_in = np.tile(np.eye(32, dtype=np.float32), (4, 1))

    in_maps = []
    for core in range(8):
        mod, h = core // 2, core % 2
        x = mods[mod]
        t2 = t2s[mod]
        xk_in = _s4(np.ascontiguousarray(x.T))
        xq_l = mods[0][sels[h]]
        xq_in = _s4(np.ascontiguousarray(xq_l.T))
        t1_in = t1_locals[h].reshape(1, TQ).astype(np.float32)
        t2p_in = np.ascontiguousarray(t2.reshape(NCH, 128).T)
        v2n_in = np.ascontiguousarray(
            x[:, :2].reshape(NCH, 128, 2).transpose(1, 0, 2)
            .reshape(128, 2 * NCH))
        v2nr_in = chop11(v2n_in)
        v2ne_in = v2n_in - v2nr_in
        v4n_in = np.zeros((128, 4 * NCH), dtype=np.float32)
        v4n_in[:, 0::4] = v2nr_in[:, 0::2]
        v4n_in[:, 1::4] = v2nr_in[:, 1::2]
        v4n_in[:, 2::4] = v2ne_in[:, 0::2]
        v4n_in[:, 3::4] = v2ne_in[:, 1::2]
        wp32_in = np.concatenate(
            [blockdiag(Wq), blockdiag(Wk), bq_in, bk_in, id4_in, t2p_in],
            axis=1)
        wpr_in = v4n_in
        in_maps.append({
            "xk": xk_in, "xq": xq_in, "wp32": wp32_in, "wpr": wpr_in,
            "t1": t1_in,
        })

    import os as _os
    trace = bool(_os.environ.get("KERNEL_TRACE"))
    res = run_bass_kernel_spmd(nc, in_maps, core_ids=list(range(8)),
                               trace=trace)
    global LAST_RESULTS
    LAST_RESULTS = res

    y = np.zeros((T, 2), dtype=np.float32)
    for core in range(8):
        mod, h = core // 2, core % 2
        zt = res.results[core]["out"]
        y[sels[h]] += zt.T
    return y[None, :, :]
